# revision 1
# baseline (speedup 1.0000x reference)
"""Deformable conv net kernel for 8 TRN2 NeuronCores (data-parallel over batch).

v3: gather-before-matmul. Per core (one batch sample):
  1. offsets via transposed 3x3 conv (out free dim = 18)      (PE)
  2. bilinear fields: corner indices + weights, pixel-major   (DVE)
  3. SWDGE pair-gather of x channel rows from host-prepared
     xT [HW, C] in DRAM (2 descs of 512B per pixel/tap)       (Pool+DMA)
  4. S^T[c,p] += gt^T @ diag(w): scale+accumulate+transpose
     in one matmul per (chunk,k,corner)                       (PE, diag on DVE/ACT)
  5. out^T[o,p] = sum_k wmain_k^T @ S_k^T + bias              (PE)
  6. host reassembles [8, 128, 64, 64] from out^T [O, HW].
"""
import os, sys

for _p in ("/opt/trn_rl_repo", "/root/.axon_site/_ro/trn_rl_repo"):
    if os.path.isdir(_p) and _p not in sys.path:
        sys.path.insert(0, _p)

import numpy as np
import ml_dtypes

import concourse.bass as bass
import concourse.mybir as mybir
from concourse import bacc, library_config
from concourse.tile import TileContext

BF16 = mybir.dt.bfloat16
F32 = mybir.dt.float32
I16 = mybir.dt.int16

B, C, H, W = 8, 128, 64, 64
O = 128
K = 3
K2 = 9
HW = H * W                 # 4096
NCH = HW // 128            # 32 pixel chunks of 128
NH = 2                     # halves of the pixel space for the gather phase
CPH = NCH // NH            # 16 chunks per half
GP = 66                    # guarded row pitch of xg
XG = (H + 2) * GP          # guarded image cols
FDIM = NCH * K2            # 288
MAGIC = float(3 * 2 ** 22)  # 1.5*2^23: keeps s+M in the ulp=1 binade

_MAX_WAITS = 1             # this walrus build rejects >1 sem wait per inst


def _split_excess_waits(nc):
    for f in nc.m.functions:
        for bb in f.blocks:
            new_insts = []
            for inst in bb.instructions:
                si = inst.sync_info
                if si is not None and si.on_wait and len(si.on_wait) > _MAX_WAITS:
                    waits = list(si.on_wait)
                    keep = waits[-_MAX_WAITS:]
                    spill = waits[:-_MAX_WAITS]
                    for j in range(0, len(spill), _MAX_WAITS):
                        chunk = spill[j:j + _MAX_WAITS]
                        nop = mybir.InstNoOp(
                            name=f"{inst.name}-wsp{j}",
                            engine=inst.engine,
                            ins=[], outs=[],
                            sync_info=mybir.SyncInfo(on_wait=chunk, on_update=[]),
                        )
                        nc.register_instruction(nop, overwrite=True)
                        new_insts.append(nop)
                    inst.sync_info = mybir.SyncInfo(
                        on_wait=keep, on_update=list(si.on_update or []))
                new_insts.append(inst)
            bb.instructions[:] = new_insts


def build_nc(act_diag_mod=6, gtbufs=5, dgbufs=128, ngsplit=2):
    nc = bacc.Bacc()
    xg_in = nc.dram_tensor("xg", [C, XG], BF16, kind="ExternalInput")
    xt_in = nc.dram_tensor("xt", [HW, C], BF16, kind="ExternalInput")
    offw_in = nc.dram_tensor("offw", [C, K2 * 18], BF16, kind="ExternalInput")
    wmain_in = nc.dram_tensor("wmain", [C, K2 * O], BF16, kind="ExternalInput")
    biaso_in = nc.dram_tensor("biaso", [128, 1], F32, kind="ExternalInput")
    ybase_in = nc.dram_tensor("ybase", [128, FDIM], F32, kind="ExternalInput")
    xbase_in = nc.dram_tensor("xbase", [128, FDIM], F32, kind="ExternalInput")
    idb_in = nc.dram_tensor("identb", [128, 128], BF16, kind="ExternalInput")
    out_dram = nc.dram_tensor("out", [O, HW], F32, kind="ExternalOutput")

    with TileContext(nc) as tc:
        with tc.tile_pool(name="cst", bufs=1) as cst, \
             tc.tile_pool(name="fld", bufs=1) as fld, \
             tc.tile_pool(name="gth", bufs=gtbufs) as gth, \
             tc.tile_pool(name="dgp", bufs=dgbufs) as dgp, \
             tc.tile_pool(name="stb", bufs=1) as stb, \
             tc.tile_pool(name="otb", bufs=2) as otb:

            nc.gpsimd.load_library(library_config.mlp)

            # Tiny SWDGE op up front: bass barriers POOL's first dynamic DMA
            # against ALL outstanding HWDGE lanes; firing it now (nothing in
            # flight) keeps that barrier off the gather critical path.
            warm = cst.tile([16, 16], BF16, name="warm")
            nc.gpsimd.dma_start(warm[:, :], xg_in[0:16, 0:16])

            # ---- constant / input loads ----
            offw_sb = cst.tile([C, K2 * 18], BF16, name="offw_sb")
            nc.sync.dma_start(offw_sb[:, :], offw_in[:, :])
            wmain_sb = cst.tile([C, K2 * O], BF16, name="wmain_sb")
            nc.sync.dma_start(wmain_sb[:, :], wmain_in[:, :])
            biaso_sb = cst.tile([128, 1], F32, name="biaso_sb")
            nc.sync.dma_start(biaso_sb[:, :], biaso_in[:, :])
            ybase_sb = cst.tile([128, FDIM], F32, name="ybase_sb")
            nc.sync.dma_start(ybase_sb[:, :], ybase_in[:, :])
            xbase_sb = cst.tile([128, FDIM], F32, name="xbase_sb")
            nc.sync.dma_start(xbase_sb[:, :], xbase_in[:, :])
            identb = cst.tile([128, 128], BF16, name="identb")
            nc.sync.dma_start(identb[:, :], idb_in[:, :])
            xg_sb = cst.tile([C, XG], BF16, name="xg_sb")
            nc.sync.dma_start(xg_sb[:, :], xg_in[:, :])

            # ---- offset conv, transposed: offT[p, c*18 + j] ----
            # lhsT = guarded x pixels for chunk c shifted by tap (ky,kx);
            # rhs = offw tap slice [C, 18]; out free dim = 18.
            offT = fld.tile([128, NCH * 18], F32, name="offT")
            xg3 = xg_sb[:, :].rearrange("c (r w) -> c r w", w=GP)
            pso_cm = tc.tile_pool(name="pso", bufs=2, space="PSUM")
            pso = pso_cm.__enter__()
            for cg in range(8):
                ps = pso.tile([128, 8 * 18], F32, name=f"offps{cg}", tag="offps")
                ps_r = ps[:, :].rearrange("p (c4 two j) -> p two c4 j", two=2, j=18)
                for c4 in range(4):
                    c = cg * 4 + c4
                    for r in range(2):
                        for k in range(K2):
                            ky, kx = k // 3, k % 3
                            lhs = xg3[:, 2 * c + r + ky, kx: kx + 64]
                            nc.tensor.matmul(
                                ps[r * 64:(r + 1) * 64,
                                   (c4 * 2 + r) * 18:(c4 * 2 + r + 1) * 18],
                                lhs,
                                offw_sb[:, k * 18:(k + 1) * 18],
                                start=(k == 0), stop=(k == K2 - 1))
                offT_r = offT[:, :].rearrange("p (c j) -> p c j", j=18)
                for r in range(2):
                    nc.vector.tensor_copy(
                        offT_r[r * 64:(r + 1) * 64, cg * 4:(cg + 1) * 4],
                        ps_r[r * 64:(r + 1) * 64, r])
            pso_cm.__exit__(None, None, None)

            # ---- bilinear fields (fp32, [128, (c,k)=288]) ----
            offT4 = offT[:, :].rearrange("p (c k two) -> p two c k", two=2, k=K2)
            yb3 = ybase_sb[:, :].rearrange("p (c k) -> p c k", k=K2)
            xb3 = xbase_sb[:, :].rearrange("p (c k) -> p c k", k=K2)

            def f3(name):
                t = fld.tile([128, FDIM], F32, name=name, tag=name)
                return t, t[:, :].rearrange("p (c k) -> p c k", k=K2)

            VA = mybir.AluOpType

            # ==== pass 1: index path only (gates the gathers) ====
            srg = {}
            for ax in ("y", "x"):
                s, s3 = f3(f"s_{ax}")
                base3 = yb3 if ax == "y" else xb3
                nc.vector.tensor_tensor(s3, offT4[:, 0 if ax == "y" else 1], base3, VA.add)
                r, r3 = f3(f"r_{ax}")
                nc.vector.tensor_scalar_add(r[:, :], s[:, :], MAGIC)
                nc.vector.tensor_scalar_add(r[:, :], r[:, :], -MAGIC)
                g, g3 = f3(f"g_{ax}")
                nc.vector.tensor_tensor(g[:, :], r[:, :], s[:, :], VA.is_gt)
                i0, _ = f3(f"i0_{ax}")
                nc.vector.tensor_tensor(i0[:, :], r[:, :], g[:, :], VA.subtract)
                srg[ax] = (s, i0)
            iy0, ix0f = srg["y"][1], srg["x"][1]
            iy1, _ = f3("i1_y")
            nc.vector.tensor_scalar_add(iy1[:, :], iy0[:, :], 1.0)
            cy = []
            for a, ii in ((0, iy0), (1, iy1)):
                cc, _ = f3(f"c_y_{a}")
                nc.vector.tensor_scalar(cc[:, :], ii[:, :], 0.0, float(H - 1),
                                        VA.max, VA.min)
                cy.append(cc)
            bx, _ = f3("bx")
            nc.vector.tensor_scalar(bx[:, :], ix0f[:, :], 0.0, float(W - 2),
                                    VA.max, VA.min)
            # pair row indices idx = cy*64 + bx
            cys = []
            for a in range(2):
                cs, _ = f3(f"cys{a}")
                nc.vector.tensor_scalar_mul(cs[:, :], cy[a][:, :], float(W))
                cys.append(cs)
            # fidx col = ((k*2+a)*NH + h)*CPH + j  (chunk c = h*CPH + j)
            fidx = fld.tile([128, 2 * FDIM], F32, name="fidx")
            fidx_r = fidx[:, :].rearrange("p (k a h j) -> p a h j k",
                                          k=K2, a=2, h=NH, j=CPH)
            for a in range(2):
                nc.vector.tensor_tensor(fidx_r[:, a],
                                        cys[a][:, :].rearrange(
                                            "p (h j k) -> p h j k",
                                            h=NH, j=CPH, k=K2),
                                        bx[:, :].rearrange(
                                            "p (h j k) -> p h j k",
                                            h=NH, j=CPH, k=K2), VA.add)
            fidxi = fld.tile([128, 2 * FDIM], I16, name="fidxi")
            nc.vector.tensor_copy(fidxi[:, :], fidx[:, :])

            # ---- fold indices into SWDGE wrapped layout (ACT copies) ----
            # idxw col = kahj*8 + f; value stream for (k,a,h): i = j*128 + p
            # -> wrapped (i%16 = p%16, i//16 = j*8 + p//16)
            # stage 1: collapse partitions 128->16 with contiguous DMAs
            # (f-major staging layout), ~51ns each
            NKAHJ = 2 * FDIM  # 576
            stg = fld.tile([128, 8 * NKAHJ], I16, name="idxstg")
            for f in range(8):
                nc.scalar.dma_start(stg[0:16, f * NKAHJ:(f + 1) * NKAHJ],
                                    fidxi[16 * f:16 * (f + 1), :])
            # stage 2: in-partition column transpose (f, kahj) -> (kahj, f)
            idxw = fld.tile([128, NKAHJ * 8], I16, name="idxw")
            stg_r = stg[:, :].rearrange("p (f kahj) -> p kahj f", f=8)
            idxw_r = idxw[:, :].rearrange("p (kahj f) -> p kahj f", f=8)
            nc.scalar.copy(idxw_r[0:16, :NKAHJ // 2], stg_r[0:16, :NKAHJ // 2])
            nc.vector.tensor_copy(idxw_r[0:16, NKAHJ // 2:],
                                  stg_r[0:16, NKAHJ // 2:])
            # replicate the 16 wrapped partitions across all 128 (ACT HWDGE)
            for f in range(1, 8):
                nc.scalar.dma_start(idxw[16 * f:16 * (f + 1), :], idxw[0:16, :])

            # ==== pass 2: weight path (overlaps the first gathers) ====
            axes_w = {}
            for ax in ("y", "x"):
                s, i0 = srg[ax]
                fr, _ = f3(f"fr_{ax}")
                nc.vector.tensor_tensor(fr[:, :], s[:, :], i0[:, :], VA.subtract)
                i1 = iy1 if ax == "y" else None
                if i1 is None:
                    i1, _ = f3("i1_x")
                    nc.vector.tensor_scalar_add(i1[:, :], i0[:, :], 1.0)
                w_m = []
                for (ii, frac_is_w) in ((i0, False), (i1, True)):
                    v, _ = f3(f"v_{ax}_{frac_is_w}")
                    nc.vector.tensor_scalar(v[:, :], ii[:, :], 0.0, None, VA.is_ge)
                    t2, _ = f3(f"t2_{ax}_{frac_is_w}")
                    nc.vector.tensor_scalar(t2[:, :], ii[:, :], float(H - 1), None, VA.is_le)
                    nc.vector.tensor_tensor(v[:, :], v[:, :], t2[:, :], VA.mult)
                    wm, _ = f3(f"wm_{ax}_{frac_is_w}")
                    if frac_is_w:
                        nc.vector.tensor_tensor(wm[:, :], fr[:, :], v[:, :], VA.mult)
                    else:
                        nc.vector.tensor_scalar(wm[:, :], fr[:, :], -1.0, 1.0,
                                                VA.mult, VA.add)
                        nc.vector.tensor_tensor(wm[:, :], wm[:, :], v[:, :], VA.mult)
                    w_m.append(wm)
                axes_w[ax] = w_m
            wy, wx = axes_w["y"], axes_w["x"]
            dif, _ = f3("dif")
            nc.vector.tensor_tensor(dif[:, :], bx[:, :], ix0f[:, :], VA.subtract)
            eqA, _ = f3("eqA")
            nc.vector.tensor_scalar(eqA[:, :], dif[:, :], 0.0, None, VA.is_equal)
            eqB, _ = f3("eqB")
            nc.vector.tensor_scalar(eqB[:, :], dif[:, :], 1.0, None, VA.is_equal)
            eqC, _ = f3("eqC")
            nc.vector.tensor_scalar(eqC[:, :], dif[:, :], -1.0, None, VA.is_equal)
            WL, _ = f3("WL")
            WR, _ = f3("WR")
            t1, _ = f3("t1")
            nc.vector.tensor_tensor(WL[:, :], wx[0][:, :], eqA[:, :], VA.mult)
            nc.vector.tensor_tensor(t1[:, :], wx[1][:, :], eqB[:, :], VA.mult)
            nc.vector.tensor_tensor(WL[:, :], WL[:, :], t1[:, :], VA.add)
            nc.vector.tensor_tensor(WR[:, :], wx[1][:, :], eqA[:, :], VA.mult)
            nc.vector.tensor_tensor(t1[:, :], wx[0][:, :], eqC[:, :], VA.mult)
            nc.vector.tensor_tensor(WR[:, :], WR[:, :], t1[:, :], VA.add)
            # weights per (a, side): wcor2[a*2+side]
            wcor2 = []
            for a in range(2):
                for sd, Wside in ((0, WL), (1, WR)):
                    wc, _ = f3(f"wc{a}{sd}")
                    nc.vector.tensor_tensor(wc[:, :], wy[a][:, :], Wside[:, :], VA.mult)
                    wcor2.append(wc)

            # ---- gather + diag-matmul accumulate + GEMM ----
            xsrc = xt_in[:, :]
            xpairs = bass.AP(tensor=xsrc.tensor, offset=xsrc.offset,
                             ap=[[C, HW - 1], [1, 2 * C]])
            psp_cm = tc.tile_pool(name="ps", bufs=2, space="PSUM")
            psp = psp_cm.__enter__()
            pso2_cm = tc.tile_pool(name="pso2", bufs=1, space="PSUM")
            pso2 = pso2_cm.__enter__()
            ndiag = 0
            for h in range(NH):
                st_sb = stb.tile([128, K2 * CPH * 128], BF16,
                                 name=f"st{h}", tag="st")
                pend = []   # deferred S^T copies: (k, q, st_ps)
                for k in range(K2):
                    gts = []
                    for a in range(2):
                        gt = gth.tile([128, CPH, 2 * C], BF16,
                                      name=f"g{h}_{k}_{a}", tag="gath")
                        base = ((k * 2 + a) * NH + h) * CPH * 8
                        ni = CPH * 128 // ngsplit
                        for g2 in range(ngsplit):
                            cpg = CPH // ngsplit
                            nc.gpsimd.dma_gather(
                                gt[:, g2 * cpg:(g2 + 1) * cpg, :], xpairs,
                                idxw[:, base + g2 * cpg * 8:
                                     base + (g2 + 1) * cpg * 8],
                                ni, ni, 2 * C, elem_step=C)
                        gts.append(gt)
                    for q in range(2):
                        st_ps = psp.tile([128, 8 * 128], F32,
                                         name=f"sp{h}_{k}_{q}", tag="stps")
                        for j8 in range(8):
                            j = q * 8 + j8
                            c = h * CPH + j
                            for a in range(2):
                                for sd in range(2):
                                    wcol = wcor2[a * 2 + sd][:, c * K2 + k:
                                                             c * K2 + k + 1]
                                    dg = dgp.tile([128, 128], BF16,
                                                  name=f"d{h}_{k}_{q}_{a}_{sd}_{j8}",
                                                  tag="diag")
                                    if act_diag_mod and ndiag % act_diag_mod == 0:
                                        nc.scalar.activation(
                                            dg[:, :], identb[:, :],
                                            mybir.ActivationFunctionType.Copy,
                                            scale=wcol)
                                    else:
                                        nc.vector.tensor_scalar_mul(
                                            dg[:, :], identb[:, :], wcol)
                                    ndiag += 1
                                    nc.tensor.matmul(
                                        st_ps[:, j8 * 128:(j8 + 1) * 128],
                                        gts[a][:, j, sd * C:(sd + 1) * C],
                                        dg[:, :],
                                        start=(a == 0 and sd == 0),
                                        stop=(a == 1 and sd == 1))
                        # defer the previous tile's copy until after this
                        # tile's diags+matmuls are queued (1-tile lookahead)
                        for kq, ps_t in pend:
                            nc.scalar.copy(
                                st_sb[:, kq * 1024:(kq + 1) * 1024], ps_t[:, :])
                        pend = [(k * 2 + q, st_ps)]
                for kq, ps_t in pend:
                    nc.scalar.copy(
                        st_sb[:, kq * 1024:(kq + 1) * 1024], ps_t[:, :])
                pend = []
                # GEMM: out^T[o, p] = sum_k wmain_k^T @ S_k^T, + bias
                ot_ps = pso2.tile([128, CPH * 128], F32, name=f"ot{h}", tag="otps")
                for j in range(CPH):
                    q, j8 = j // 8, j % 8
                    for k in range(K2):
                        nc.tensor.matmul(
                            ot_ps[:, j * 128:(j + 1) * 128],
                            wmain_sb[:, k * O:(k + 1) * O],
                            st_sb[:, (k * 2 + q) * 1024 + j8 * 128:
                                  (k * 2 + q) * 1024 + (j8 + 1) * 128],
                            start=(k == 0), stop=(k == K2 - 1))
                ot_sb = otb.tile([128, CPH * 128], F32, name=f"ots{h}", tag="ots")
                nc.scalar.activation(ot_sb[:, :], ot_ps[:, :],
                                     mybir.ActivationFunctionType.Identity,
                                     bias=biaso_sb[:, 0:1])
                nc.sync.dma_start(out_dram[:, h * CPH * 128:(h + 1) * CPH * 128],
                                  ot_sb[:, :])
            pso2_cm.__exit__(None, None, None)
            psp_cm.__exit__(None, None, None)

    nc.compile()
    _split_excess_waits(nc)
    return nc


_NC_CACHE = None


def _get_nc():
    global _NC_CACHE
    if _NC_CACHE is None:
        _NC_CACHE = build_nc()
    return _NC_CACHE


def _host_inputs(x, offset_w, offset_b, weight, bias):
    bf = ml_dtypes.bfloat16
    offw = np.ascontiguousarray(
        offset_w.reshape(18, C, K2).transpose(1, 2, 0).reshape(C, K2 * 18)).astype(bf)
    wmain = np.ascontiguousarray(
        weight.reshape(O, C, K2).transpose(1, 2, 0).reshape(C, K2 * O)).astype(bf)
    biaso = bias.reshape(128, 1).astype(np.float32)
    pi = np.arange(128)
    cc = np.arange(NCH)
    kk = np.arange(K2)
    pix = cc[None, :, None] * 128 + pi[:, None, None]          # [128, 32, 1]
    ob = offset_b.reshape(K2, 2).astype(np.float32)
    ybase = (pix // W - 1 + (kk // 3)[None, None, :]
             + ob[None, None, :, 0]).reshape(128, FDIM).astype(np.float32)
    xbase = (pix % W - 1 + (kk % 3)[None, None, :]
             + ob[None, None, :, 1]).reshape(128, FDIM).astype(np.float32)
    identb = np.eye(128, dtype=bf)
    shared = dict(offw=offw, wmain=wmain, biaso=biaso,
                  ybase=ybase, xbase=xbase, identb=identb)
    maps = []
    for b in range(B):
        m = dict(shared)
        xb = x[b].reshape(C, H, W)
        P = np.zeros((C, H + 2, GP), np.float32)
        P[:, 1:H + 1, 1:W + 1] = xb
        m["xg"] = P.reshape(C, XG).astype(bf)
        m["xt"] = np.ascontiguousarray(x[b].reshape(C, HW).T).astype(bf)
        maps.append(m)
    return maps


def kernel(x, offset_w, offset_b, weight, bias):
    from concourse.bass_utils import run_bass_kernel_spmd
    nc = _get_nc()
    in_maps = _host_inputs(np.asarray(x, np.float32), np.asarray(offset_w, np.float32),
                           np.asarray(offset_b, np.float32),
                           np.asarray(weight, np.float32), np.asarray(bias, np.float32))
    res = run_bass_kernel_spmd(nc, in_maps, core_ids=list(range(B)))
    out = np.stack([np.asarray(res.results[b]["out"], np.float32).reshape(O, H, W)
                    for b in range(B)])
    return out



# revision 43
# speedup vs baseline: 1.1261x; 1.1261x over previous
"""Deformable conv net kernel for 8 TRN2 NeuronCores (data-parallel over batch).

v4.3: quad-gather, h-split head, quartered tail. Per core (one sample):
  1. offsets via transposed 3x3 conv (out free dim = 18)          (PE)
  2. bilinear fields: ONE quad index + 4 redistributed corner
     weights per (pixel, tap); index path h-split so the first
     gathers launch after only half the field work               (DVE)
  3. SWDGE gather of host-prepared quad rows Q[y*W+x] = the
     2x2 corner patch (4C = 1KB contiguous): ONE descriptor
     per (pixel, tap)                                            (Pool+DMA)
  4. S^T[c,p] += gt_corner^T @ diag(w_corner): 4 matmuls/chunk
     into PSUM; diag tiles built 32-at-a-time against a wide
     identity with a dup-paired broadcast weight AP (DVE 2x)     (PE, DVE)
  5. out^T[o,p] += wmain_k^T @ S_k^T accumulated per-k into a
     persistent PSUM tile (k-interleaved GEMM, tiny tail)        (PE)
  6. bf16 out store; host reassembles [8, 128, 64, 64].
"""
import os, sys

for _p in ("/opt/trn_rl_repo", "/root/.axon_site/_ro/trn_rl_repo"):
    if os.path.isdir(_p) and _p not in sys.path:
        sys.path.insert(0, _p)

import numpy as np
import ml_dtypes

import concourse.bass as bass
import concourse.mybir as mybir
from concourse import bacc, library_config
from concourse.tile import TileContext

BF16 = mybir.dt.bfloat16
F32 = mybir.dt.float32
I16 = mybir.dt.int16

B, C, H, W = 8, 128, 64, 64
O = 128
K = 3
K2 = 9
HW = H * W                 # 4096
NCH = HW // 128            # 32 pixel chunks of 128
NH = 2                     # halves of the pixel space
CPH = NCH // NH            # 16 chunks per half
HD = CPH * K2              # 144 field cols per half
GP = 66                    # guarded row pitch of xg
XG = (H + 2) * GP          # guarded image cols
FDIM = NCH * K2            # 288
QE = 4 * C                 # quad payload elems (512)
MAGIC = float(3 * 2 ** 22)  # 1.5*2^23: keeps s+M in the ulp=1 binade

_MAX_WAITS = 1             # this walrus build rejects >1 sem wait per inst


def _split_excess_waits(nc):
    for f in nc.m.functions:
        for bb in f.blocks:
            new_insts = []
            for inst in bb.instructions:
                si = inst.sync_info
                if si is not None and si.on_wait and len(si.on_wait) > _MAX_WAITS:
                    waits = list(si.on_wait)
                    keep = waits[-_MAX_WAITS:]
                    spill = waits[:-_MAX_WAITS]
                    for j in range(0, len(spill), _MAX_WAITS):
                        chunk = spill[j:j + _MAX_WAITS]
                        nop = mybir.InstNoOp(
                            name=f"{inst.name}-wsp{j}",
                            engine=inst.engine,
                            ins=[], outs=[],
                            sync_info=mybir.SyncInfo(on_wait=chunk, on_update=[]),
                        )
                        nc.register_instruction(nop, overwrite=True)
                        new_insts.append(nop)
                    inst.sync_info = mybir.SyncInfo(
                        on_wait=keep, on_update=list(si.on_update or []))
                new_insts.append(inst)
            bb.instructions[:] = new_insts


def _ap(t, offset_cols, dims):
    """Manual AP over a tile's slice (linearized element strides)."""
    b = t[:, :]
    return bass.AP(tensor=b.tensor, offset=b.offset + offset_cols, ap=dims)


def build_nc(gt_bufs=3, dg_bufs=4):
    nc = bacc.Bacc()
    xg_in = nc.dram_tensor("xg", [C, XG], BF16, kind="ExternalInput")
    xq_in = nc.dram_tensor("xq", [HW, QE], BF16, kind="ExternalInput")
    offw_in = nc.dram_tensor("offw", [C, K2 * 18], BF16, kind="ExternalInput")
    wmain_in = nc.dram_tensor("wmain", [C, K2 * O], BF16, kind="ExternalInput")
    biaso_in = nc.dram_tensor("biaso", [128, 1], F32, kind="ExternalInput")
    ybase_in = nc.dram_tensor("ybase", [128, FDIM], F32, kind="ExternalInput")
    xbase_in = nc.dram_tensor("xbase", [128, FDIM], F32, kind="ExternalInput")
    idw_in = nc.dram_tensor("idwide", [128, 32 * 128], BF16, kind="ExternalInput")
    out_dram = nc.dram_tensor("out", [O, HW], BF16, kind="ExternalOutput")

    VA = mybir.AluOpType
    XG1 = 35 * GP   # xg cols needed by offconv chunk-groups 0-3

    with TileContext(nc) as tc:
        with tc.tile_pool(name="cst", bufs=1) as cst, \
             tc.tile_pool(name="fld", bufs=1) as fld, \
             tc.tile_pool(name="gth", bufs=gt_bufs) as gth, \
             tc.tile_pool(name="dgp", bufs=dg_bufs) as dgp, \
             tc.tile_pool(name="stb", bufs=3) as stb, \
             tc.tile_pool(name="otb", bufs=2) as otb:

            nc.gpsimd.load_library(library_config.mlp)

            # Tiny SWDGE op up front: bass barriers POOL's first dynamic DMA
            # against ALL outstanding HWDGE lanes; firing it now (nothing in
            # flight) keeps that barrier off the gather critical path.
            warm = cst.tile([16, 16], BF16, name="warm")
            nc.gpsimd.dma_start(warm[:, :], xg_in[0:16, 0:16])

            # ---- constant / input loads (ACT HWDGE, critical-path first) ----
            xg_sb = cst.tile([C, XG], BF16, name="xg_sb")
            nc.scalar.dma_start(xg_sb[:, 0:XG1], xg_in[:, 0:XG1])
            offw_sb = cst.tile([C, K2 * 18], BF16, name="offw_sb")
            nc.scalar.dma_start(offw_sb[:, :], offw_in[:, :])
            ybase_sb = cst.tile([128, FDIM], F32, name="ybase_sb")
            nc.scalar.dma_start(ybase_sb[:, :], ybase_in[:, :])
            xbase_sb = cst.tile([128, FDIM], F32, name="xbase_sb")
            nc.scalar.dma_start(xbase_sb[:, :], xbase_in[:, :])
            nc.scalar.dma_start(xg_sb[:, XG1:], xg_in[:, XG1:])
            idwide = cst.tile([128, 32 * 128], BF16, name="idwide")
            nc.scalar.dma_start(idwide[:, :], idw_in[:, :])
            wmain_sb = cst.tile([C, K2 * O], BF16, name="wmain_sb")
            nc.scalar.dma_start(wmain_sb[:, :], wmain_in[:, :])
            biaso_sb = cst.tile([128, 1], F32, name="biaso_sb")
            nc.scalar.dma_start(biaso_sb[:, :], biaso_in[:, :])

            # ---- offset conv, transposed: offT[p, c*18 + j] ----
            offT = fld.tile([128, NCH * 18], F32, name="offT")
            xg3 = xg_sb[:, :].rearrange("c (r w) -> c r w", w=GP)
            offT_r = offT[:, :].rearrange("p (c j) -> p c j", j=18)
            pso_cm = tc.tile_pool(name="pso", bufs=2, space="PSUM")
            pso = pso_cm.__enter__()

            def offconv(cg):
                ps = pso.tile([128, 8 * 18], F32, name=f"offps{cg}", tag="offps")
                ps_r = ps[:, :].rearrange("p (c4 two j) -> p two c4 j", two=2, j=18)
                for c4 in range(4):
                    c = cg * 4 + c4
                    for r in range(2):
                        for k in range(K2):
                            ky, kx = k // 3, k % 3
                            lhs = xg3[:, 2 * c + r + ky, kx: kx + 64]
                            nc.tensor.matmul(
                                ps[r * 64:(r + 1) * 64,
                                   (c4 * 2 + r) * 18:(c4 * 2 + r + 1) * 18],
                                lhs,
                                offw_sb[:, k * 18:(k + 1) * 18],
                                start=(k == 0), stop=(k == K2 - 1))
                for r in range(2):
                    nc.vector.tensor_copy(
                        offT_r[r * 64:(r + 1) * 64, cg * 4:(cg + 1) * 4],
                        ps_r[r * 64:(r + 1) * 64, r])

            for cg in range(4):
                offconv(cg)

            # ---- bilinear fields (fp32, [128, (c,k)=288] c-major) ----
            # c-major col = c*9 + k, so half hh = contiguous cols [hh*HD,(hh+1)*HD)
            offT4 = offT[:, :].rearrange("p (c k two) -> p two c k", two=2, k=K2)
            yb3 = ybase_sb[:, :].rearrange("p (c k) -> p c k", k=K2)
            xb3 = xbase_sb[:, :].rearrange("p (c k) -> p c k", k=K2)

            tiles = {}

            def f3(name):
                if name not in tiles:
                    tiles[name] = fld.tile([128, FDIM], F32, name=name, tag=name)
                return tiles[name]

            def fsl(t, hh):  # [128, HD] half slice, c-major
                return t[:, hh * HD:(hh + 1) * HD]

            def f3c(t, hh):  # half slice viewed (c, k)
                return t[:, hh * HD:(hh + 1) * HD].rearrange(
                    "p (c k) -> p c k", k=K2)

            # index staging: fidx/fidxi h-major col = hh*HD + k*16 + j
            fidx = fld.tile([128, FDIM], F32, name="fidx")
            fidxi = fld.tile([128, FDIM], I16, name="fidxi")
            # wrapped indices: stg[p16, hh*1152 + f*HD + k*16 + j] (f-major)
            # idxw[p, (hh*9 + k)*128 + j*8 + f]
            stg = fld.tile([128, 2 * 8 * HD], I16, name="idxstg")
            idxw = fld.tile([128, 2 * 8 * HD], I16, name="idxw")

            srg = {}

            def pass1(hh):
                for ax in ("y", "x"):
                    s, r, g, i0, cc = (f3(f"s_{ax}"), f3(f"r_{ax}"),
                                       f3(f"g_{ax}"), f3(f"i0_{ax}"),
                                       f3(f"c_{ax}"))
                    base3 = yb3 if ax == "y" else xb3
                    nc.vector.tensor_tensor(
                        f3c(s, hh), offT4[:, 0 if ax == "y" else 1,
                                          hh * CPH:(hh + 1) * CPH],
                        base3[:, hh * CPH:(hh + 1) * CPH], VA.add)
                    nc.vector.tensor_scalar_add(fsl(r, hh), fsl(s, hh), MAGIC)
                    nc.vector.tensor_scalar_add(fsl(r, hh), fsl(r, hh), -MAGIC)
                    nc.vector.tensor_tensor(fsl(g, hh), fsl(r, hh), fsl(s, hh),
                                            VA.is_gt)
                    nc.vector.tensor_tensor(fsl(i0, hh), fsl(r, hh), fsl(g, hh),
                                            VA.subtract)
                    nc.vector.tensor_scalar(fsl(cc, hh), fsl(i0, hh), 0.0,
                                            float(H - 2), VA.max, VA.min)
                    srg[ax] = (s, i0, cc)
                cys = f3("cys")
                nc.vector.tensor_scalar_mul(fsl(cys, hh), fsl(srg["y"][2], hh),
                                            float(W))
                # fidx[:, hh*HD + k*16 + j] = cys + cx  (h-major, k-blocks)
                out_ap = _ap(fidx, hh * HD,
                             [[FDIM, 128], [CPH, K2], [1, CPH]])
                in_y = _ap(f3("cys"), hh * HD, [[FDIM, 128], [1, K2], [K2, CPH]])
                in_x = _ap(f3("c_x"), hh * HD, [[FDIM, 128], [1, K2], [K2, CPH]])
                nc.vector.tensor_tensor(out_ap, in_y, in_x, VA.add)
                nc.vector.tensor_copy(fsl(fidxi, hh), fsl(fidx, hh))

            def wrap(hh, engines):
                # collapse 128->16 partitions: 8 small DMAs, split between
                # HWDGE issuers and Pool's software DGE (idle before gathers)
                for f in range(8):
                    eng = engines[f % len(engines)]
                    eng.dma_start(
                        stg[0:16, hh * 8 * HD + f * HD:
                            hh * 8 * HD + (f + 1) * HD],
                        fidxi[16 * f:16 * (f + 1), hh * HD:(hh + 1) * HD])
                # in-partition transpose (f, kj) -> (kj, f), two pipelined
                # pieces so each only waits its half of the collapse DMAs
                for piece in range(2):
                    o_ap = _ap(idxw, hh * 8 * HD + piece * 4,
                               [[2 * 8 * HD, 16], [128, K2], [8, CPH], [1, 4]])
                    i_ap = _ap(stg, hh * 8 * HD + piece * 4 * HD,
                               [[2 * 8 * HD, 16], [CPH, K2], [1, CPH], [HD, 4]])
                    if hh == 0:
                        (nc.scalar.copy if piece == 0
                         else nc.vector.tensor_copy)(o_ap, i_ap)
                    else:
                        nc.vector.tensor_copy(o_ap, i_ap)
                # replicate to all 128 partitions (7 parallel DMAs)
                for f in range(1, 8):
                    eng = engines[f % len(engines)]
                    eng.dma_start(idxw[16 * f:16 * (f + 1),
                                       hh * 8 * HD:(hh + 1) * 8 * HD],
                                  idxw[0:16, hh * 8 * HD:(hh + 1) * 8 * HD])

            # corner-weight pack, dup-paired for DVE 2x broadcast reads:
            # col2 = (k*128 + c*4 + corner)*2 + d, corner order TL,TR,BL,BR
            wpack2 = fld.tile([128, 2 * 4 * FDIM], BF16, name="wpack2")

            def pass2(hh):
                WW = {}
                for ax in ("y", "x"):
                    s, i0, cc = srg[ax]
                    fr, v0, v1, t2 = (f3(f"fr_{ax}"), f3(f"v0_{ax}"),
                                      f3(f"v1_{ax}"), f3(f"t2_{ax}"))
                    nc.vector.tensor_tensor(fsl(fr, hh), fsl(s, hh),
                                            fsl(i0, hh), VA.subtract)
                    nc.vector.tensor_scalar(fsl(v0, hh), fsl(i0, hh), 0.0,
                                            None, VA.is_ge)
                    nc.vector.tensor_scalar(fsl(t2, hh), fsl(i0, hh),
                                            float(H - 1), None, VA.is_le)
                    nc.vector.tensor_tensor(fsl(v0, hh), fsl(v0, hh),
                                            fsl(t2, hh), VA.mult)
                    nc.vector.tensor_scalar(fsl(v1, hh), fsl(i0, hh), -1.0,
                                            None, VA.is_ge)
                    nc.vector.tensor_scalar(fsl(t2, hh), fsl(i0, hh),
                                            float(H - 2), None, VA.is_le)
                    nc.vector.tensor_tensor(fsl(v1, hh), fsl(v1, hh),
                                            fsl(t2, hh), VA.mult)
                    w1v, w0v = f3(f"w1v_{ax}"), f3(f"w0v_{ax}")
                    nc.vector.tensor_tensor(fsl(w1v, hh), fsl(fr, hh),
                                            fsl(v1, hh), VA.mult)
                    nc.vector.tensor_scalar(fsl(w0v, hh), fsl(fr, hh), -1.0,
                                            1.0, VA.mult, VA.add)
                    nc.vector.tensor_tensor(fsl(w0v, hh), fsl(w0v, hh),
                                            fsl(v0, hh), VA.mult)
                    dif = f3(f"dif_{ax}")
                    nc.vector.tensor_tensor(fsl(dif, hh), fsl(cc, hh),
                                            fsl(i0, hh), VA.subtract)
                    eq0, eqP, eqM = (f3(f"eq0_{ax}"), f3(f"eqP_{ax}"),
                                     f3(f"eqM_{ax}"))
                    nc.vector.tensor_scalar(fsl(eq0, hh), fsl(dif, hh), 0.0,
                                            None, VA.is_equal)
                    nc.vector.tensor_scalar(fsl(eqP, hh), fsl(dif, hh), 1.0,
                                            None, VA.is_equal)
                    nc.vector.tensor_scalar(fsl(eqM, hh), fsl(dif, hh), -1.0,
                                            None, VA.is_equal)
                    t1, Wa, Wb = f3(f"t1_{ax}"), f3(f"Wa_{ax}"), f3(f"Wb_{ax}")
                    nc.vector.tensor_tensor(fsl(Wa, hh), fsl(w0v, hh),
                                            fsl(eq0, hh), VA.mult)
                    nc.vector.tensor_tensor(fsl(t1, hh), fsl(w1v, hh),
                                            fsl(eqP, hh), VA.mult)
                    nc.vector.tensor_tensor(fsl(Wa, hh), fsl(Wa, hh),
                                            fsl(t1, hh), VA.add)
                    nc.vector.tensor_tensor(fsl(Wb, hh), fsl(w1v, hh),
                                            fsl(eq0, hh), VA.mult)
                    nc.vector.tensor_tensor(fsl(t1, hh), fsl(w0v, hh),
                                            fsl(eqM, hh), VA.mult)
                    nc.vector.tensor_tensor(fsl(Wb, hh), fsl(Wb, hh),
                                            fsl(t1, hh), VA.add)
                    WW[ax] = (Wa, Wb)
                # products -> wpack2 (k-major quad layout, dup-paired)
                for co, (wy_t, wx_t) in enumerate(
                        ((WW["y"][0], WW["x"][0]), (WW["y"][0], WW["x"][1]),
                         (WW["y"][1], WW["x"][0]), (WW["y"][1], WW["x"][1]))):
                    o_ap = _ap(wpack2, hh * 128 + co * 2,
                               [[8 * FDIM, 128], [256, K2], [8, CPH], [1, 2]])
                    iy = _ap(wy_t, hh * HD,
                             [[FDIM, 128], [1, K2], [K2, CPH], [0, 2]])
                    ix = _ap(wx_t, hh * HD,
                             [[FDIM, 128], [1, K2], [K2, CPH], [0, 2]])
                    nc.vector.tensor_tensor(o_ap, iy, ix, VA.mult)

            def wides_g(h, k, pool, tag="dgw"):
                dgs = []
                for q in range(2):
                    if tag is None:
                        dgw = pool.tile([128, 32 * 128], BF16,
                                        name=f"dl{h}_{k}_{q}")
                    else:
                        dgw = pool.tile([128, 32 * 128], BF16,
                                        name=f"d{h}_{k}_{q}", tag=tag)
                    col0 = k * 128 + (h * CPH + q * 8) * 4
                    in2 = _ap(wpack2, 2 * col0,
                              [[2 * 4 * FDIM, 128], [2, 32], [0, 64], [1, 2]])
                    nc.vector.tensor_tensor(dgw[:, :], idwide[:, :], in2,
                                            VA.mult)
                    dgs.append(dgw)
                return dgs

            # ---- h=0 index path, then the rest staged around it ----
            pass1(0)
            wrap(0, [nc.sync, nc.scalar, nc.gpsimd, nc.scalar])
            pass2(0)
            # prebuild the first slots' diag tiles so the earliest matmuls
            # don't wait behind the h=1 field work on DVE
            prebuilt = {(0, 0): wides_g(0, 0, dgp), (0, 1): wides_g(0, 1, dgp)}
            for cg in range(4, 8):
                offconv(cg)
            pass1(1)
            wrap(1, [nc.sync])
            # more h=0 wides ahead of the (slack-rich) h=1 weight pass; the
            # dgw pool rotation paces these against the consuming matmuls
            for kq in range(2, 6):
                prebuilt[(0, kq)] = wides_g(0, kq, dgp)
            pass2(1)
            pso_cm.__exit__(None, None, None)

            # ---- gather + diag-matmul accumulate + k-interleaved GEMM ----
            xq0 = xq_in[:, :]
            xq_ap = bass.AP(tensor=xq0.tensor, offset=xq0.offset,
                            ap=[[QE, HW], [1, QE]])
            psp_cm = tc.tile_pool(name="ps", bufs=4, space="PSUM")
            psp = psp_cm.__enter__()
            pso2_cm = tc.tile_pool(name="pso2", bufs=1, space="PSUM")
            pso2 = pso2_cm.__enter__()

            for h in range(NH):
                ot_ps = pso2.tile([128, CPH * 128], F32, name=f"ot{h}", tag="otps")

                def copies(kp, stp):
                    st_sb = stb.tile([128, CPH * 128], BF16,
                                     name=f"st{h}_{kp}", tag="st")
                    for qq in range(4):
                        # k=7's copies run during the tail: use DVE (idle by
                        # then) so they don't serialize with the quarter
                        # copies on ACT
                        if last_h and kp == K2 - 2:
                            nc.vector.tensor_copy(
                                st_sb[:, qq * 512:(qq + 1) * 512], stp[qq][:, :])
                        else:
                            nc.scalar.copy(st_sb[:, qq * 512:(qq + 1) * 512],
                                           stp[qq][:, :])
                    return st_sb

                def gemm(kp, st_sb, jbs=range(4)):
                    # full-PSUM-bank output regions (512 cols fp32): one
                    # open accumulation group per bank across k=0..8
                    for jb in jbs:
                        nc.tensor.matmul(
                            ot_ps[:, jb * 512:(jb + 1) * 512],
                            wmain_sb[:, kp * O:(kp + 1) * O],
                            st_sb[:, jb * 512:(jb + 1) * 512],
                            start=(kp == 0), stop=(kp == K2 - 1))

                def diag_mms(k, gt, dgs, qq, st_ps):
                    for j4 in range(4):
                        j = qq * 4 + j4
                        dg = dgs[j // 8]
                        blk = (j % 8) * 4
                        for co in range(4):
                            nc.tensor.matmul(
                                st_ps[:, j4 * 128:(j4 + 1) * 128],
                                gt[:, j, co * C:(co + 1) * C],
                                dg[:, (blk + co) * 128:(blk + co + 1) * 128],
                                start=(co == 0), stop=(co == 3))

                last_h = (h == NH - 1)
                # pre-build the final slot's diag tiles in dedicated buffers
                # so the tail never waits on DVE or dgw-buffer rotation
                dgs_last = wides_g(h, K2 - 1, cst, tag=None) if last_h else None

                ot_sb = otb.tile([128, CPH * 128], BF16, name=f"ots{h}", tag="ots")
                pend = []   # [(k, st_ps quarters)] awaiting copies
                gpend = []  # [(k, st_sb)] awaiting GEMM
                for k in range(K2):
                    last_slot = last_h and k == K2 - 1
                    base = (h * K2 + k) * 128
                    if not last_slot:
                        gt = gth.tile([128, CPH, QE], BF16,
                                      name=f"g{h}_{k}", tag="gath")
                        # SWDGE ring caps a gather at 1024 descriptors
                        for g2 in range(2):
                            nc.gpsimd.dma_gather(
                                gt[:, g2 * 8:(g2 + 1) * 8, :], xq_ap,
                                idxw[:, base + g2 * 64:base + (g2 + 1) * 64],
                                1024, 1024, QE)
                    else:
                        # last slot: two half tiles so matmuls start on the
                        # first half while the second is still in flight
                        gts = []
                        for g2 in range(2):
                            gth_t = gth.tile([128, 8, QE], BF16,
                                             name=f"g{h}_{k}_{g2}", tag="gath")
                            nc.gpsimd.dma_gather(
                                gth_t[:, :, :], xq_ap,
                                idxw[:, base + g2 * 64:base + (g2 + 1) * 64],
                                1024, 1024, QE)
                            gts.append(gth_t)

                    # drain the pipeline: copies for k-1, GEMM for k-2
                    for kp, stp in pend:
                        gpend.append((kp, copies(kp, stp)))
                    pend = []
                    while len(gpend) > 1:
                        gemm(*gpend.pop(0))

                    if last_slot:
                        dgs = dgs_last
                    elif (h, k) in prebuilt:
                        dgs = prebuilt.pop((h, k))
                    else:
                        dgs = wides_g(h, k, dgp)
                    if not last_slot:
                        st_q = []
                        for qq in range(4):
                            st_ps = psp.tile([128, 4 * 128], F32,
                                             name=f"sp{h}_{k}_{qq}", tag="stps")
                            diag_mms(k, gt, dgs, qq, st_ps)
                            st_q.append(st_ps)
                        pend.append((k, st_q))
                    else:
                        # tail: k-1's GEMM interleaves after the first
                        # quarters so PE never waits on the just-issued
                        # copies. k=8's stop-GEMMs must follow k-1's in PE
                        # program order (PSUM stop = last write per bank).
                        st_sb = stb.tile([128, CPH * 128], BF16,
                                         name=f"st{h}_{k}", tag="st")

                        def act_store(half):
                            sl = slice(half * 1024, (half + 1) * 1024)
                            nc.scalar.activation(
                                ot_sb[:, sl], ot_ps[:, sl],
                                mybir.ActivationFunctionType.Identity,
                                bias=biaso_sb[:, 0:1])
                            nc.sync.dma_start(
                                out_dram[:, h * CPH * 128 + half * 1024:
                                         h * CPH * 128 + (half + 1) * 1024],
                                ot_sb[:, sl])

                        for qq in range(4):
                            gtx = gts[qq // 2]
                            st_ps = psp.tile([128, 4 * 128], F32,
                                             name=f"sp{h}_{k}_{qq}", tag="stps")
                            for j4 in range(4):
                                j = qq * 4 + j4
                                dg = dgs[j // 8]
                                blk = (j % 8) * 4
                                for co in range(4):
                                    nc.tensor.matmul(
                                        st_ps[:, j4 * 128:(j4 + 1) * 128],
                                        gtx[:, j % 8, co * C:(co + 1) * C],
                                        dg[:, (blk + co) * 128:
                                           (blk + co + 1) * 128],
                                        start=(co == 0), stop=(co == 3))
                            if qq % 2 == 0:
                                nc.scalar.copy(
                                    st_sb[:, qq * 512:(qq + 1) * 512],
                                    st_ps[:, :])
                            else:
                                nc.vector.tensor_copy(
                                    st_sb[:, qq * 512:(qq + 1) * 512],
                                    st_ps[:, :])
                            if qq == 1:
                                for kp, st_sb_p in gpend:
                                    gemm(kp, st_sb_p)   # k-1, copies now done
                                gpend = []
                                gemm(k, st_sb, jbs=[0])
                            elif qq >= 2:
                                gemm(k, st_sb, jbs=[qq - 1])
                            if qq == 2:
                                act_store(0)
                        gemm(k, st_sb, jbs=[3])
                        act_store(1)

                if not last_h:
                    # tail for h=0: flush k=8 + act/store in halves
                    for kp, stp in pend:
                        gpend.append((kp, copies(kp, stp)))
                    for kp, st_sb_p in gpend:
                        gemm(kp, st_sb_p)
                    pend, gpend = [], []
                    for half in range(2):
                        sl = slice(half * 1024, (half + 1) * 1024)
                        nc.scalar.activation(
                            ot_sb[:, sl], ot_ps[:, sl],
                            mybir.ActivationFunctionType.Identity,
                            bias=biaso_sb[:, 0:1])
                        nc.sync.dma_start(
                            out_dram[:, h * CPH * 128 + half * 1024:
                                     h * CPH * 128 + (half + 1) * 1024],
                            ot_sb[:, sl])
            pso2_cm.__exit__(None, None, None)
            psp_cm.__exit__(None, None, None)

    nc.compile()
    _split_excess_waits(nc)
    return nc


_NC_CACHE = None


def _get_nc():
    global _NC_CACHE
    if _NC_CACHE is None:
        _NC_CACHE = build_nc()
    return _NC_CACHE


def _host_inputs(x, offset_w, offset_b, weight, bias):
    bf = ml_dtypes.bfloat16
    offw = np.ascontiguousarray(
        offset_w.reshape(18, C, K2).transpose(1, 2, 0).reshape(C, K2 * 18)).astype(bf)
    wmain = np.ascontiguousarray(
        weight.reshape(O, C, K2).transpose(1, 2, 0).reshape(C, K2 * O)).astype(bf)
    biaso = bias.reshape(128, 1).astype(np.float32)
    pi = np.arange(128)
    cc = np.arange(NCH)
    kk = np.arange(K2)
    pix = cc[None, :, None] * 128 + pi[:, None, None]          # [128, 32, 1]
    ob = offset_b.reshape(K2, 2).astype(np.float32)
    ybase = (pix // W - 1 + (kk // 3)[None, None, :]
             + ob[None, None, :, 0]).reshape(128, FDIM).astype(np.float32)
    xbase = (pix % W - 1 + (kk % 3)[None, None, :]
             + ob[None, None, :, 1]).reshape(128, FDIM).astype(np.float32)
    idwide = np.ascontiguousarray(
        np.broadcast_to(np.eye(128, dtype=bf)[:, None, :],
                        (128, 32, 128))).reshape(128, 32 * 128)
    shared = dict(offw=offw, wmain=wmain, biaso=biaso,
                  ybase=ybase, xbase=xbase, idwide=idwide)
    maps = []
    for b in range(B):
        m = dict(shared)
        xb = x[b].reshape(C, H, W)
        P = np.zeros((C, H + 2, GP), np.float32)
        P[:, 1:H + 1, 1:W + 1] = xb
        m["xg"] = P.reshape(C, XG).astype(bf)
        # quad tensor: Q[y*W+x] = [x(y,x,:), x(y,x+1,:), x(y+1,x,:), x(y+1,x+1,:)]
        xp = np.zeros((C, H + 1, W + 1), np.float32)
        xp[:, :H, :W] = xb
        q = np.stack([xp[:, :H, :W], xp[:, :H, 1:W + 1],
                      xp[:, 1:H + 1, :W], xp[:, 1:H + 1, 1:W + 1]], axis=0)
        m["xq"] = np.ascontiguousarray(
            q.transpose(2, 3, 0, 1).reshape(HW, QE)).astype(bf)
        maps.append(m)
    return maps


def kernel(x, offset_w, offset_b, weight, bias):
    from concourse.bass_utils import run_bass_kernel_spmd
    nc = _get_nc()
    in_maps = _host_inputs(np.asarray(x, np.float32), np.asarray(offset_w, np.float32),
                           np.asarray(offset_b, np.float32),
                           np.asarray(weight, np.float32), np.asarray(bias, np.float32))
    res = run_bass_kernel_spmd(nc, in_maps, core_ids=list(range(B)))
    out = np.stack([np.asarray(res.results[b]["out"], np.float32).reshape(O, H, W)
                    for b in range(B)])
    return out


# revision 46
# speedup vs baseline: 1.1332x; 1.0063x over previous
"""Deformable conv net kernel for 8 TRN2 NeuronCores (data-parallel over batch).

v4.3: quad-gather, h-split head, quartered tail. Per core (one sample):
  1. offsets via transposed 3x3 conv (out free dim = 18)          (PE)
  2. bilinear fields: ONE quad index + 4 redistributed corner
     weights per (pixel, tap); index path h-split so the first
     gathers launch after only half the field work               (DVE)
  3. SWDGE gather of host-prepared quad rows Q[y*W+x] = the
     2x2 corner patch (4C = 1KB contiguous): ONE descriptor
     per (pixel, tap)                                            (Pool+DMA)
  4. S^T[c,p] += gt_corner^T @ diag(w_corner): 4 matmuls/chunk
     into PSUM; diag tiles built 32-at-a-time against a wide
     identity with a dup-paired broadcast weight AP (DVE 2x)     (PE, DVE)
  5. out^T[o,p] += wmain_k^T @ S_k^T accumulated per-k into a
     persistent PSUM tile (k-interleaved GEMM, tiny tail)        (PE)
  6. bf16 out store; host reassembles [8, 128, 64, 64].
"""
import os, sys

for _p in ("/opt/trn_rl_repo", "/root/.axon_site/_ro/trn_rl_repo"):
    if os.path.isdir(_p) and _p not in sys.path:
        sys.path.insert(0, _p)

import numpy as np
import ml_dtypes

import concourse.bass as bass
import concourse.mybir as mybir
from concourse import bacc, library_config
from concourse.tile import TileContext

BF16 = mybir.dt.bfloat16
F32 = mybir.dt.float32
I16 = mybir.dt.int16

B, C, H, W = 8, 128, 64, 64
O = 128
K = 3
K2 = 9
HW = H * W                 # 4096
NCH = HW // 128            # 32 pixel chunks of 128
NH = 2                     # halves of the pixel space
CPH = NCH // NH            # 16 chunks per half
HD = CPH * K2              # 144 field cols per half
GP = 66                    # guarded row pitch of xg
XG = (H + 2) * GP          # guarded image cols
FDIM = NCH * K2            # 288
QE = 4 * C                 # quad payload elems (512)
MAGIC = float(3 * 2 ** 22)  # 1.5*2^23: keeps s+M in the ulp=1 binade

_MAX_WAITS = 1             # this walrus build rejects >1 sem wait per inst


def _split_excess_waits(nc):
    for f in nc.m.functions:
        for bb in f.blocks:
            new_insts = []
            for inst in bb.instructions:
                si = inst.sync_info
                if si is not None and si.on_wait and len(si.on_wait) > _MAX_WAITS:
                    waits = list(si.on_wait)
                    keep = waits[-_MAX_WAITS:]
                    spill = waits[:-_MAX_WAITS]
                    for j in range(0, len(spill), _MAX_WAITS):
                        chunk = spill[j:j + _MAX_WAITS]
                        nop = mybir.InstNoOp(
                            name=f"{inst.name}-wsp{j}",
                            engine=inst.engine,
                            ins=[], outs=[],
                            sync_info=mybir.SyncInfo(on_wait=chunk, on_update=[]),
                        )
                        nc.register_instruction(nop, overwrite=True)
                        new_insts.append(nop)
                    inst.sync_info = mybir.SyncInfo(
                        on_wait=keep, on_update=list(si.on_update or []))
                new_insts.append(inst)
            bb.instructions[:] = new_insts


def _ap(t, offset_cols, dims):
    """Manual AP over a tile's slice (linearized element strides)."""
    b = t[:, :]
    return bass.AP(tensor=b.tensor, offset=b.offset + offset_cols, ap=dims)


def build_nc(gt_bufs=3, dg_bufs=4):
    nc = bacc.Bacc()
    xg_in = nc.dram_tensor("xg", [C, XG], BF16, kind="ExternalInput")
    xq_in = nc.dram_tensor("xq", [HW, QE], BF16, kind="ExternalInput")
    offw_in = nc.dram_tensor("offw", [C, K2 * 18], BF16, kind="ExternalInput")
    wmain_in = nc.dram_tensor("wmain", [C, K2 * O], BF16, kind="ExternalInput")
    biaso_in = nc.dram_tensor("biaso", [128, 1], F32, kind="ExternalInput")
    ybase_in = nc.dram_tensor("ybase", [128, FDIM], F32, kind="ExternalInput")
    xbase_in = nc.dram_tensor("xbase", [128, FDIM], F32, kind="ExternalInput")
    idw_in = nc.dram_tensor("idwide", [128, 32 * 128], BF16, kind="ExternalInput")
    out_dram = nc.dram_tensor("out", [O, HW], BF16, kind="ExternalOutput")

    VA = mybir.AluOpType
    XG1 = 35 * GP   # xg cols needed by offconv chunk-groups 0-3

    with TileContext(nc) as tc:
        with tc.tile_pool(name="cst", bufs=1) as cst, \
             tc.tile_pool(name="fld", bufs=1) as fld, \
             tc.tile_pool(name="gth", bufs=gt_bufs) as gth, \
             tc.tile_pool(name="dgp", bufs=dg_bufs) as dgp, \
             tc.tile_pool(name="stb", bufs=3) as stb, \
             tc.tile_pool(name="otb", bufs=2) as otb:

            nc.gpsimd.load_library(library_config.mlp)

            # Tiny SWDGE op up front: bass barriers POOL's first dynamic DMA
            # against ALL outstanding HWDGE lanes; firing it now (nothing in
            # flight) keeps that barrier off the gather critical path.
            warm = cst.tile([16, 16], BF16, name="warm")
            nc.gpsimd.dma_start(warm[:, :], xg_in[0:16, 0:16])

            # ---- constant / input loads (ACT HWDGE, critical-path first) ----
            xg_sb = cst.tile([C, XG], BF16, name="xg_sb")
            nc.scalar.dma_start(xg_sb[:, 0:XG1], xg_in[:, 0:XG1])
            offw_sb = cst.tile([C, K2 * 18], BF16, name="offw_sb")
            nc.scalar.dma_start(offw_sb[:, :], offw_in[:, :])
            ybase_sb = cst.tile([128, FDIM], F32, name="ybase_sb")
            nc.scalar.dma_start(ybase_sb[:, :], ybase_in[:, :])
            xbase_sb = cst.tile([128, FDIM], F32, name="xbase_sb")
            nc.scalar.dma_start(xbase_sb[:, :], xbase_in[:, :])
            nc.scalar.dma_start(xg_sb[:, XG1:], xg_in[:, XG1:])
            idwide = cst.tile([128, 32 * 128], BF16, name="idwide")
            nc.scalar.dma_start(idwide[:, :], idw_in[:, :])
            wmain_sb = cst.tile([C, K2 * O], BF16, name="wmain_sb")
            nc.scalar.dma_start(wmain_sb[:, :], wmain_in[:, :])
            biaso_sb = cst.tile([128, 1], F32, name="biaso_sb")
            nc.scalar.dma_start(biaso_sb[:, :], biaso_in[:, :])

            # ---- offset conv, transposed: offT[p, c*18 + j] ----
            offT = fld.tile([128, NCH * 18], F32, name="offT")
            xg3 = xg_sb[:, :].rearrange("c (r w) -> c r w", w=GP)
            offT_r = offT[:, :].rearrange("p (c j) -> p c j", j=18)
            pso_cm = tc.tile_pool(name="pso", bufs=2, space="PSUM")
            pso = pso_cm.__enter__()

            def offconv(cg):
                ps = pso.tile([128, 8 * 18], F32, name=f"offps{cg}", tag="offps")
                ps_r = ps[:, :].rearrange("p (c4 two j) -> p two c4 j", two=2, j=18)
                for c4 in range(4):
                    c = cg * 4 + c4
                    for r in range(2):
                        for k in range(K2):
                            ky, kx = k // 3, k % 3
                            lhs = xg3[:, 2 * c + r + ky, kx: kx + 64]
                            nc.tensor.matmul(
                                ps[r * 64:(r + 1) * 64,
                                   (c4 * 2 + r) * 18:(c4 * 2 + r + 1) * 18],
                                lhs,
                                offw_sb[:, k * 18:(k + 1) * 18],
                                start=(k == 0), stop=(k == K2 - 1))
                for r in range(2):
                    nc.vector.tensor_copy(
                        offT_r[r * 64:(r + 1) * 64, cg * 4:(cg + 1) * 4],
                        ps_r[r * 64:(r + 1) * 64, r])

            for cg in range(4):
                offconv(cg)

            # ---- bilinear fields (fp32, [128, (c,k)=288] c-major) ----
            # c-major col = c*9 + k, so half hh = contiguous cols [hh*HD,(hh+1)*HD)
            offT4 = offT[:, :].rearrange("p (c k two) -> p two c k", two=2, k=K2)
            yb3 = ybase_sb[:, :].rearrange("p (c k) -> p c k", k=K2)
            xb3 = xbase_sb[:, :].rearrange("p (c k) -> p c k", k=K2)

            tiles = {}

            def f3(name):
                if name not in tiles:
                    tiles[name] = fld.tile([128, FDIM], F32, name=name, tag=name)
                return tiles[name]

            def fsl(t, hh):  # [128, HD] half slice, c-major
                return t[:, hh * HD:(hh + 1) * HD]

            def f3c(t, hh):  # half slice viewed (c, k)
                return t[:, hh * HD:(hh + 1) * HD].rearrange(
                    "p (c k) -> p c k", k=K2)

            # index staging: fidx/fidxi h-major col = hh*HD + k*16 + j
            fidx = fld.tile([128, FDIM], F32, name="fidx")
            fidxi = fld.tile([128, FDIM], I16, name="fidxi")
            # wrapped indices: stg[p16, hh*1152 + f*HD + k*16 + j] (f-major)
            # idxw[p, (hh*9 + k)*128 + j*8 + f]
            stg = fld.tile([128, 2 * 8 * HD], I16, name="idxstg")
            idxw = fld.tile([128, 2 * 8 * HD], I16, name="idxw")

            srg = {}

            def pass1(hh):
                for ax in ("y", "x"):
                    s, r, g, i0, cc = (f3(f"s_{ax}"), f3(f"r_{ax}"),
                                       f3(f"g_{ax}"), f3(f"i0_{ax}"),
                                       f3(f"c_{ax}"))
                    base3 = yb3 if ax == "y" else xb3
                    nc.vector.tensor_tensor(
                        f3c(s, hh), offT4[:, 0 if ax == "y" else 1,
                                          hh * CPH:(hh + 1) * CPH],
                        base3[:, hh * CPH:(hh + 1) * CPH], VA.add)
                    nc.vector.tensor_scalar(fsl(r, hh), fsl(s, hh), MAGIC,
                                            -MAGIC, VA.add, VA.add)
                    nc.vector.tensor_tensor(fsl(g, hh), fsl(r, hh), fsl(s, hh),
                                            VA.is_gt)
                    nc.vector.tensor_tensor(fsl(i0, hh), fsl(r, hh), fsl(g, hh),
                                            VA.subtract)
                    nc.vector.tensor_scalar(fsl(cc, hh), fsl(i0, hh), 0.0,
                                            float(H - 2), VA.max, VA.min)
                    srg[ax] = (s, i0, cc)
                # fidx[:, hh*HD + k*16 + j] = cy*W + cx  (h-major, k-blocks)
                out_ap = _ap(fidx, hh * HD,
                             [[FDIM, 128], [CPH, K2], [1, CPH]])
                in_y = _ap(f3("c_y"), hh * HD, [[FDIM, 128], [1, K2], [K2, CPH]])
                in_x = _ap(f3("c_x"), hh * HD, [[FDIM, 128], [1, K2], [K2, CPH]])
                nc.vector.scalar_tensor_tensor(out_ap, in_y, float(W), in_x,
                                               VA.mult, VA.add)
                nc.vector.tensor_copy(fsl(fidxi, hh), fsl(fidx, hh))

            def wrap(hh, engines):
                # collapse 128->16 partitions: 8 small DMAs, split between
                # HWDGE issuers and Pool's software DGE (idle before gathers)
                for f in range(8):
                    eng = engines[f % len(engines)]
                    eng.dma_start(
                        stg[0:16, hh * 8 * HD + f * HD:
                            hh * 8 * HD + (f + 1) * HD],
                        fidxi[16 * f:16 * (f + 1), hh * HD:(hh + 1) * HD])
                # in-partition transpose (f, kj) -> (kj, f), two pipelined
                # pieces so each only waits its half of the collapse DMAs
                for piece in range(2):
                    o_ap = _ap(idxw, hh * 8 * HD + piece * 4,
                               [[2 * 8 * HD, 16], [128, K2], [8, CPH], [1, 4]])
                    i_ap = _ap(stg, hh * 8 * HD + piece * 4 * HD,
                               [[2 * 8 * HD, 16], [CPH, K2], [1, CPH], [HD, 4]])
                    if hh == 0:
                        (nc.scalar.copy if piece == 0
                         else nc.vector.tensor_copy)(o_ap, i_ap)
                    else:
                        nc.vector.tensor_copy(o_ap, i_ap)
                # replicate to all 128 partitions (7 parallel DMAs)
                for f in range(1, 8):
                    eng = engines[f % len(engines)]
                    eng.dma_start(idxw[16 * f:16 * (f + 1),
                                       hh * 8 * HD:(hh + 1) * 8 * HD],
                                  idxw[0:16, hh * 8 * HD:(hh + 1) * 8 * HD])

            # corner-weight pack, dup-paired for DVE 2x broadcast reads:
            # col2 = (k*128 + c*4 + corner)*2 + d, corner order TL,TR,BL,BR
            wpack2 = fld.tile([128, 2 * 4 * FDIM], BF16, name="wpack2")

            def pass2(hh):
                WW = {}
                for ax in ("y", "x"):
                    s, i0, cc = srg[ax]
                    fr, v0, v1, t2 = (f3(f"fr_{ax}"), f3(f"v0_{ax}"),
                                      f3(f"v1_{ax}"), f3(f"t2_{ax}"))
                    nc.vector.tensor_tensor(fsl(fr, hh), fsl(s, hh),
                                            fsl(i0, hh), VA.subtract)
                    nc.vector.tensor_scalar(fsl(v0, hh), fsl(i0, hh), 0.0,
                                            None, VA.is_ge)
                    nc.vector.tensor_scalar(fsl(t2, hh), fsl(i0, hh),
                                            float(H - 1), None, VA.is_le)
                    nc.vector.tensor_tensor(fsl(v0, hh), fsl(v0, hh),
                                            fsl(t2, hh), VA.mult)
                    nc.vector.tensor_scalar(fsl(v1, hh), fsl(i0, hh), -1.0,
                                            None, VA.is_ge)
                    nc.vector.tensor_scalar(fsl(t2, hh), fsl(i0, hh),
                                            float(H - 2), None, VA.is_le)
                    nc.vector.tensor_tensor(fsl(v1, hh), fsl(v1, hh),
                                            fsl(t2, hh), VA.mult)
                    w1v, w0v = f3(f"w1v_{ax}"), f3(f"w0v_{ax}")
                    nc.vector.tensor_tensor(fsl(w1v, hh), fsl(fr, hh),
                                            fsl(v1, hh), VA.mult)
                    nc.vector.tensor_scalar(fsl(w0v, hh), fsl(fr, hh), -1.0,
                                            1.0, VA.mult, VA.add)
                    nc.vector.tensor_tensor(fsl(w0v, hh), fsl(w0v, hh),
                                            fsl(v0, hh), VA.mult)
                    dif = f3(f"dif_{ax}")
                    nc.vector.tensor_tensor(fsl(dif, hh), fsl(cc, hh),
                                            fsl(i0, hh), VA.subtract)
                    eq0, eqP, eqM = (f3(f"eq0_{ax}"), f3(f"eqP_{ax}"),
                                     f3(f"eqM_{ax}"))
                    nc.vector.tensor_scalar(fsl(eq0, hh), fsl(dif, hh), 0.0,
                                            None, VA.is_equal)
                    nc.vector.tensor_scalar(fsl(eqP, hh), fsl(dif, hh), 1.0,
                                            None, VA.is_equal)
                    nc.vector.tensor_scalar(fsl(eqM, hh), fsl(dif, hh), -1.0,
                                            None, VA.is_equal)
                    t1, Wa, Wb = f3(f"t1_{ax}"), f3(f"Wa_{ax}"), f3(f"Wb_{ax}")
                    nc.vector.tensor_tensor(fsl(Wa, hh), fsl(w0v, hh),
                                            fsl(eq0, hh), VA.mult)
                    nc.vector.tensor_tensor(fsl(t1, hh), fsl(w1v, hh),
                                            fsl(eqP, hh), VA.mult)
                    nc.vector.tensor_tensor(fsl(Wa, hh), fsl(Wa, hh),
                                            fsl(t1, hh), VA.add)
                    nc.vector.tensor_tensor(fsl(Wb, hh), fsl(w1v, hh),
                                            fsl(eq0, hh), VA.mult)
                    nc.vector.tensor_tensor(fsl(t1, hh), fsl(w0v, hh),
                                            fsl(eqM, hh), VA.mult)
                    nc.vector.tensor_tensor(fsl(Wb, hh), fsl(Wb, hh),
                                            fsl(t1, hh), VA.add)
                    WW[ax] = (Wa, Wb)
                # products -> wpack2 (k-major quad layout, dup-paired)
                for co, (wy_t, wx_t) in enumerate(
                        ((WW["y"][0], WW["x"][0]), (WW["y"][0], WW["x"][1]),
                         (WW["y"][1], WW["x"][0]), (WW["y"][1], WW["x"][1]))):
                    o_ap = _ap(wpack2, hh * 128 + co * 2,
                               [[8 * FDIM, 128], [256, K2], [8, CPH], [1, 2]])
                    iy = _ap(wy_t, hh * HD,
                             [[FDIM, 128], [1, K2], [K2, CPH], [0, 2]])
                    ix = _ap(wx_t, hh * HD,
                             [[FDIM, 128], [1, K2], [K2, CPH], [0, 2]])
                    nc.vector.tensor_tensor(o_ap, iy, ix, VA.mult)

            def wides_g(h, k, pool, tag="dgw"):
                dgs = []
                for q in range(2):
                    if tag is None:
                        dgw = pool.tile([128, 32 * 128], BF16,
                                        name=f"dl{h}_{k}_{q}")
                    else:
                        dgw = pool.tile([128, 32 * 128], BF16,
                                        name=f"d{h}_{k}_{q}", tag=tag)
                    col0 = k * 128 + (h * CPH + q * 8) * 4
                    in2 = _ap(wpack2, 2 * col0,
                              [[2 * 4 * FDIM, 128], [2, 32], [0, 64], [1, 2]])
                    nc.vector.tensor_tensor(dgw[:, :], idwide[:, :], in2,
                                            VA.mult)
                    dgs.append(dgw)
                return dgs

            # ---- h=0 index path, then the rest staged around it ----
            pass1(0)
            wrap(0, [nc.sync, nc.scalar, nc.gpsimd, nc.scalar])
            pass2(0)
            # prebuild the first slots' diag tiles so the earliest matmuls
            # don't wait behind the h=1 field work on DVE
            prebuilt = {(0, 0): wides_g(0, 0, dgp), (0, 1): wides_g(0, 1, dgp)}
            for cg in range(4, 8):
                offconv(cg)
            pass1(1)
            wrap(1, [nc.sync])
            # more h=0 wides ahead of the (slack-rich) h=1 weight pass; the
            # dgw pool rotation paces these against the consuming matmuls
            for kq in range(2, 6):
                prebuilt[(0, kq)] = wides_g(0, kq, dgp)
            pass2(1)
            for kq in range(6, 9):
                prebuilt[(0, kq)] = wides_g(0, kq, dgp)
            prebuilt[(1, 0)] = wides_g(1, 0, dgp)
            pso_cm.__exit__(None, None, None)

            # ---- gather + diag-matmul accumulate + k-interleaved GEMM ----
            xq0 = xq_in[:, :]
            xq_ap = bass.AP(tensor=xq0.tensor, offset=xq0.offset,
                            ap=[[QE, HW], [1, QE]])
            psp_cm = tc.tile_pool(name="ps", bufs=4, space="PSUM")
            psp = psp_cm.__enter__()
            pso2_cm = tc.tile_pool(name="pso2", bufs=1, space="PSUM")
            pso2 = pso2_cm.__enter__()

            for h in range(NH):
                ot_ps = pso2.tile([128, CPH * 128], F32, name=f"ot{h}", tag="otps")

                def copies(kp, stp):
                    st_sb = stb.tile([128, CPH * 128], BF16,
                                     name=f"st{h}_{kp}", tag="st")
                    for qq in range(4):
                        # k=7's copies run during the tail: use DVE (idle by
                        # then) so they don't serialize with the quarter
                        # copies on ACT
                        if last_h and kp == K2 - 2:
                            nc.vector.tensor_copy(
                                st_sb[:, qq * 512:(qq + 1) * 512], stp[qq][:, :])
                        else:
                            nc.scalar.copy(st_sb[:, qq * 512:(qq + 1) * 512],
                                           stp[qq][:, :])
                    return st_sb

                def gemm(kp, st_sb, jbs=range(4)):
                    # full-PSUM-bank output regions (512 cols fp32): one
                    # open accumulation group per bank across k=0..8
                    for jb in jbs:
                        nc.tensor.matmul(
                            ot_ps[:, jb * 512:(jb + 1) * 512],
                            wmain_sb[:, kp * O:(kp + 1) * O],
                            st_sb[:, jb * 512:(jb + 1) * 512],
                            start=(kp == 0), stop=(kp == K2 - 1))

                def diag_mms(k, gt, dgs, qq, st_ps):
                    for j4 in range(4):
                        j = qq * 4 + j4
                        dg = dgs[j // 8]
                        blk = (j % 8) * 4
                        for co in range(4):
                            nc.tensor.matmul(
                                st_ps[:, j4 * 128:(j4 + 1) * 128],
                                gt[:, j, co * C:(co + 1) * C],
                                dg[:, (blk + co) * 128:(blk + co + 1) * 128],
                                start=(co == 0), stop=(co == 3))

                last_h = (h == NH - 1)
                # pre-build the final slot's diag tiles in dedicated buffers
                # so the tail never waits on DVE or dgw-buffer rotation
                dgs_last = wides_g(h, K2 - 1, cst, tag=None) if last_h else None

                ot_sb = otb.tile([128, CPH * 128], BF16, name=f"ots{h}", tag="ots")
                pend = []   # [(k, st_ps quarters)] awaiting copies
                gpend = []  # [(k, st_sb)] awaiting GEMM
                for k in range(K2):
                    last_slot = last_h and k == K2 - 1
                    base = (h * K2 + k) * 128
                    if not last_slot:
                        gt = gth.tile([128, CPH, QE], BF16,
                                      name=f"g{h}_{k}", tag="gath")
                        # SWDGE ring caps a gather at 1024 descriptors
                        for g2 in range(2):
                            nc.gpsimd.dma_gather(
                                gt[:, g2 * 8:(g2 + 1) * 8, :], xq_ap,
                                idxw[:, base + g2 * 64:base + (g2 + 1) * 64],
                                1024, 1024, QE)
                    else:
                        # last slot: two half tiles so matmuls start on the
                        # first half while the second is still in flight
                        gts = []
                        for g2 in range(2):
                            gth_t = gth.tile([128, 8, QE], BF16,
                                             name=f"g{h}_{k}_{g2}", tag="gath")
                            nc.gpsimd.dma_gather(
                                gth_t[:, :, :], xq_ap,
                                idxw[:, base + g2 * 64:base + (g2 + 1) * 64],
                                1024, 1024, QE)
                            gts.append(gth_t)

                    # drain the pipeline: copies for k-1, GEMM for k-2
                    for kp, stp in pend:
                        gpend.append((kp, copies(kp, stp)))
                    pend = []
                    while len(gpend) > 1:
                        gemm(*gpend.pop(0))

                    if last_slot:
                        dgs = dgs_last
                    elif (h, k) in prebuilt:
                        dgs = prebuilt.pop((h, k))
                    else:
                        dgs = wides_g(h, k, dgp)
                    if not last_slot:
                        st_q = []
                        for qq in range(4):
                            st_ps = psp.tile([128, 4 * 128], F32,
                                             name=f"sp{h}_{k}_{qq}", tag="stps")
                            diag_mms(k, gt, dgs, qq, st_ps)
                            st_q.append(st_ps)
                        pend.append((k, st_q))
                    else:
                        # tail: k-1's GEMM interleaves after the first
                        # quarters so PE never waits on the just-issued
                        # copies. k=8's stop-GEMMs must follow k-1's in PE
                        # program order (PSUM stop = last write per bank).
                        st_sb = stb.tile([128, CPH * 128], BF16,
                                         name=f"st{h}_{k}", tag="st")

                        def act_store(half):
                            sl = slice(half * 1024, (half + 1) * 1024)
                            nc.scalar.activation(
                                ot_sb[:, sl], ot_ps[:, sl],
                                mybir.ActivationFunctionType.Identity,
                                bias=biaso_sb[:, 0:1])
                            nc.sync.dma_start(
                                out_dram[:, h * CPH * 128 + half * 1024:
                                         h * CPH * 128 + (half + 1) * 1024],
                                ot_sb[:, sl])

                        for qq in range(4):
                            gtx = gts[qq // 2]
                            st_ps = psp.tile([128, 4 * 128], F32,
                                             name=f"sp{h}_{k}_{qq}", tag="stps")
                            for j4 in range(4):
                                j = qq * 4 + j4
                                dg = dgs[j // 8]
                                blk = (j % 8) * 4
                                for co in range(4):
                                    nc.tensor.matmul(
                                        st_ps[:, j4 * 128:(j4 + 1) * 128],
                                        gtx[:, j % 8, co * C:(co + 1) * C],
                                        dg[:, (blk + co) * 128:
                                           (blk + co + 1) * 128],
                                        start=(co == 0), stop=(co == 3))
                            if qq % 2 == 0:
                                nc.scalar.copy(
                                    st_sb[:, qq * 512:(qq + 1) * 512],
                                    st_ps[:, :])
                            else:
                                nc.vector.tensor_copy(
                                    st_sb[:, qq * 512:(qq + 1) * 512],
                                    st_ps[:, :])
                            if qq == 1:
                                for kp, st_sb_p in gpend:
                                    gemm(kp, st_sb_p)   # k-1, copies now done
                                gpend = []
                                gemm(k, st_sb, jbs=[0])
                            elif qq >= 2:
                                gemm(k, st_sb, jbs=[qq - 1])
                            if qq == 2:
                                act_store(0)
                        gemm(k, st_sb, jbs=[3])
                        act_store(1)

                if not last_h:
                    # tail for h=0: flush k=8 + act/store in halves
                    for kp, stp in pend:
                        gpend.append((kp, copies(kp, stp)))
                    for kp, st_sb_p in gpend:
                        gemm(kp, st_sb_p)
                    pend, gpend = [], []
                    for half in range(2):
                        sl = slice(half * 1024, (half + 1) * 1024)
                        nc.scalar.activation(
                            ot_sb[:, sl], ot_ps[:, sl],
                            mybir.ActivationFunctionType.Identity,
                            bias=biaso_sb[:, 0:1])
                        nc.sync.dma_start(
                            out_dram[:, h * CPH * 128 + half * 1024:
                                     h * CPH * 128 + (half + 1) * 1024],
                            ot_sb[:, sl])
            pso2_cm.__exit__(None, None, None)
            psp_cm.__exit__(None, None, None)

    nc.compile()
    _split_excess_waits(nc)
    return nc


_NC_CACHE = None


def _get_nc():
    global _NC_CACHE
    if _NC_CACHE is None:
        _NC_CACHE = build_nc()
    return _NC_CACHE


def _host_inputs(x, offset_w, offset_b, weight, bias):
    bf = ml_dtypes.bfloat16
    offw = np.ascontiguousarray(
        offset_w.reshape(18, C, K2).transpose(1, 2, 0).reshape(C, K2 * 18)).astype(bf)
    wmain = np.ascontiguousarray(
        weight.reshape(O, C, K2).transpose(1, 2, 0).reshape(C, K2 * O)).astype(bf)
    biaso = bias.reshape(128, 1).astype(np.float32)
    pi = np.arange(128)
    cc = np.arange(NCH)
    kk = np.arange(K2)
    pix = cc[None, :, None] * 128 + pi[:, None, None]          # [128, 32, 1]
    ob = offset_b.reshape(K2, 2).astype(np.float32)
    ybase = (pix // W - 1 + (kk // 3)[None, None, :]
             + ob[None, None, :, 0]).reshape(128, FDIM).astype(np.float32)
    xbase = (pix % W - 1 + (kk % 3)[None, None, :]
             + ob[None, None, :, 1]).reshape(128, FDIM).astype(np.float32)
    idwide = np.ascontiguousarray(
        np.broadcast_to(np.eye(128, dtype=bf)[:, None, :],
                        (128, 32, 128))).reshape(128, 32 * 128)
    shared = dict(offw=offw, wmain=wmain, biaso=biaso,
                  ybase=ybase, xbase=xbase, idwide=idwide)
    maps = []
    for b in range(B):
        m = dict(shared)
        xb = x[b].reshape(C, H, W)
        P = np.zeros((C, H + 2, GP), np.float32)
        P[:, 1:H + 1, 1:W + 1] = xb
        m["xg"] = P.reshape(C, XG).astype(bf)
        # quad tensor: Q[y*W+x] = [x(y,x,:), x(y,x+1,:), x(y+1,x,:), x(y+1,x+1,:)]
        xp = np.zeros((C, H + 1, W + 1), np.float32)
        xp[:, :H, :W] = xb
        q = np.stack([xp[:, :H, :W], xp[:, :H, 1:W + 1],
                      xp[:, 1:H + 1, :W], xp[:, 1:H + 1, 1:W + 1]], axis=0)
        m["xq"] = np.ascontiguousarray(
            q.transpose(2, 3, 0, 1).reshape(HW, QE)).astype(bf)
        maps.append(m)
    return maps


def kernel(x, offset_w, offset_b, weight, bias):
    from concourse.bass_utils import run_bass_kernel_spmd
    nc = _get_nc()
    in_maps = _host_inputs(np.asarray(x, np.float32), np.asarray(offset_w, np.float32),
                           np.asarray(offset_b, np.float32),
                           np.asarray(weight, np.float32), np.asarray(bias, np.float32))
    res = run_bass_kernel_spmd(nc, in_maps, core_ids=list(range(B)))
    out = np.stack([np.asarray(res.results[b]["out"], np.float32).reshape(O, H, W)
                    for b in range(B)])
    return out


# revision 49
# speedup vs baseline: 1.1368x; 1.0032x over previous
"""Deformable conv net kernel for 8 TRN2 NeuronCores (data-parallel over batch).

v4.3: quad-gather, h-split head, quartered tail. Per core (one sample):
  1. offsets via transposed 3x3 conv (out free dim = 18)          (PE)
  2. bilinear fields: ONE quad index + 4 redistributed corner
     weights per (pixel, tap); index path h-split so the first
     gathers launch after only half the field work               (DVE)
  3. SWDGE gather of host-prepared quad rows Q[y*W+x] = the
     2x2 corner patch (4C = 1KB contiguous): ONE descriptor
     per (pixel, tap)                                            (Pool+DMA)
  4. S^T[c,p] += gt_corner^T @ diag(w_corner): 4 matmuls/chunk
     into PSUM; diag tiles built 32-at-a-time against a wide
     identity with a dup-paired broadcast weight AP (DVE 2x)     (PE, DVE)
  5. out^T[o,p] += wmain_k^T @ S_k^T accumulated per-k into a
     persistent PSUM tile (k-interleaved GEMM, tiny tail)        (PE)
  6. bf16 out store; host reassembles [8, 128, 64, 64].
"""
import os, sys

for _p in ("/opt/trn_rl_repo", "/root/.axon_site/_ro/trn_rl_repo"):
    if os.path.isdir(_p) and _p not in sys.path:
        sys.path.insert(0, _p)

import numpy as np
import ml_dtypes

import concourse.bass as bass
import concourse.mybir as mybir
from concourse import bacc, library_config
from concourse.tile import TileContext

BF16 = mybir.dt.bfloat16
F32 = mybir.dt.float32
I16 = mybir.dt.int16

B, C, H, W = 8, 128, 64, 64
O = 128
K = 3
K2 = 9
HW = H * W                 # 4096
NCH = HW // 128            # 32 pixel chunks of 128
NH = 2                     # halves of the pixel space
CPH = NCH // NH            # 16 chunks per half
HD = CPH * K2              # 144 field cols per half
GP = 66                    # guarded row pitch of xg
XG = (H + 2) * GP          # guarded image cols
FDIM = NCH * K2            # 288
QE = 4 * C                 # quad payload elems (512)
MAGIC = float(3 * 2 ** 22)  # 1.5*2^23: keeps s+M in the ulp=1 binade

_MAX_WAITS = 1             # this walrus build rejects >1 sem wait per inst


def _split_excess_waits(nc):
    for f in nc.m.functions:
        for bb in f.blocks:
            new_insts = []
            for inst in bb.instructions:
                si = inst.sync_info
                if si is not None and si.on_wait and len(si.on_wait) > _MAX_WAITS:
                    waits = list(si.on_wait)
                    keep = waits[-_MAX_WAITS:]
                    spill = waits[:-_MAX_WAITS]
                    for j in range(0, len(spill), _MAX_WAITS):
                        chunk = spill[j:j + _MAX_WAITS]
                        nop = mybir.InstNoOp(
                            name=f"{inst.name}-wsp{j}",
                            engine=inst.engine,
                            ins=[], outs=[],
                            sync_info=mybir.SyncInfo(on_wait=chunk, on_update=[]),
                        )
                        nc.register_instruction(nop, overwrite=True)
                        new_insts.append(nop)
                    inst.sync_info = mybir.SyncInfo(
                        on_wait=keep, on_update=list(si.on_update or []))
                new_insts.append(inst)
            bb.instructions[:] = new_insts


def _ap(t, offset_cols, dims):
    """Manual AP over a tile's slice (linearized element strides)."""
    b = t[:, :]
    return bass.AP(tensor=b.tensor, offset=b.offset + offset_cols, ap=dims)


def build_nc(gt_bufs=3, dg_bufs=4):
    nc = bacc.Bacc()
    xg_in = nc.dram_tensor("xg", [C, XG], BF16, kind="ExternalInput")
    xq_in = nc.dram_tensor("xq", [HW, QE], BF16, kind="ExternalInput")
    offw_in = nc.dram_tensor("offw", [C, K2 * 18], BF16, kind="ExternalInput")
    wmain_in = nc.dram_tensor("wmain", [C, K2 * O], BF16, kind="ExternalInput")
    biaso_in = nc.dram_tensor("biaso", [128, 1], F32, kind="ExternalInput")
    ybase_in = nc.dram_tensor("ybase", [128, FDIM], F32, kind="ExternalInput")
    xbase_in = nc.dram_tensor("xbase", [128, FDIM], F32, kind="ExternalInput")
    idw_in = nc.dram_tensor("idwide", [128, 32 * 128], BF16, kind="ExternalInput")
    out_dram = nc.dram_tensor("out", [O, HW], BF16, kind="ExternalOutput")

    VA = mybir.AluOpType
    XG1 = 35 * GP   # xg cols needed by offconv chunk-groups 0-3

    with TileContext(nc) as tc:
        with tc.tile_pool(name="cst", bufs=1) as cst, \
             tc.tile_pool(name="fld", bufs=1) as fld, \
             tc.tile_pool(name="gth", bufs=gt_bufs) as gth, \
             tc.tile_pool(name="dgp", bufs=dg_bufs) as dgp, \
             tc.tile_pool(name="stb", bufs=3) as stb, \
             tc.tile_pool(name="otb", bufs=2) as otb:

            nc.gpsimd.load_library(library_config.mlp)

            # Tiny SWDGE op up front: bass barriers POOL's first dynamic DMA
            # against ALL outstanding HWDGE lanes; firing it now (nothing in
            # flight) keeps that barrier off the gather critical path.
            warm = cst.tile([16, 16], BF16, name="warm")
            nc.gpsimd.dma_start(warm[:, :], xg_in[0:16, 0:16])

            # ---- constant / input loads (ACT HWDGE, critical-path first) ----
            xg_sb = cst.tile([C, XG], BF16, name="xg_sb")
            nc.scalar.dma_start(xg_sb[:, 0:XG1], xg_in[:, 0:XG1])
            offw_sb = cst.tile([C, K2 * 18], BF16, name="offw_sb")
            nc.scalar.dma_start(offw_sb[:, :], offw_in[:, :])
            ybase_sb = cst.tile([128, FDIM], F32, name="ybase_sb")
            nc.scalar.dma_start(ybase_sb[:, :], ybase_in[:, :])
            xbase_sb = cst.tile([128, FDIM], F32, name="xbase_sb")
            nc.scalar.dma_start(xbase_sb[:, :], xbase_in[:, :])
            nc.scalar.dma_start(xg_sb[:, XG1:], xg_in[:, XG1:])
            idwide = cst.tile([128, 32 * 128], BF16, name="idwide")
            nc.scalar.dma_start(idwide[:, :], idw_in[:, :])
            wmain_sb = cst.tile([C, K2 * O], BF16, name="wmain_sb")
            nc.scalar.dma_start(wmain_sb[:, :], wmain_in[:, :])
            biaso_sb = cst.tile([128, 1], F32, name="biaso_sb")
            nc.scalar.dma_start(biaso_sb[:, :], biaso_in[:, :])

            # ---- offset conv, transposed: offT[p, c*18 + j] ----
            offT = fld.tile([128, NCH * 18], F32, name="offT")
            xg3 = xg_sb[:, :].rearrange("c (r w) -> c r w", w=GP)
            offT_r = offT[:, :].rearrange("p (c j) -> p c j", j=18)
            pso_cm = tc.tile_pool(name="pso", bufs=2, space="PSUM")
            pso = pso_cm.__enter__()

            def offconv(cg):
                ps = pso.tile([128, 8 * 18], F32, name=f"offps{cg}", tag="offps")
                ps_r = ps[:, :].rearrange("p (c4 two j) -> p two c4 j", two=2, j=18)
                for c4 in range(4):
                    c = cg * 4 + c4
                    for r in range(2):
                        for k in range(K2):
                            ky, kx = k // 3, k % 3
                            lhs = xg3[:, 2 * c + r + ky, kx: kx + 64]
                            nc.tensor.matmul(
                                ps[r * 64:(r + 1) * 64,
                                   (c4 * 2 + r) * 18:(c4 * 2 + r + 1) * 18],
                                lhs,
                                offw_sb[:, k * 18:(k + 1) * 18],
                                start=(k == 0), stop=(k == K2 - 1))
                for r in range(2):
                    nc.vector.tensor_copy(
                        offT_r[r * 64:(r + 1) * 64, cg * 4:(cg + 1) * 4],
                        ps_r[r * 64:(r + 1) * 64, r])

            for cg in range(4):
                offconv(cg)

            # ---- bilinear fields (fp32, [128, (c,k)=288] c-major) ----
            # c-major col = c*9 + k, so half hh = contiguous cols [hh*HD,(hh+1)*HD)
            offT4 = offT[:, :].rearrange("p (c k two) -> p two c k", two=2, k=K2)
            yb3 = ybase_sb[:, :].rearrange("p (c k) -> p c k", k=K2)
            xb3 = xbase_sb[:, :].rearrange("p (c k) -> p c k", k=K2)

            tiles = {}

            def f3(name):
                if name not in tiles:
                    tiles[name] = fld.tile([128, FDIM], F32, name=name, tag=name)
                return tiles[name]

            def fsl(t, hh):  # [128, HD] half slice, c-major
                return t[:, hh * HD:(hh + 1) * HD]

            def f3c(t, hh):  # half slice viewed (c, k)
                return t[:, hh * HD:(hh + 1) * HD].rearrange(
                    "p (c k) -> p c k", k=K2)

            # index staging: fidx/fidxi h-major col = hh*HD + k*16 + j
            fidx = fld.tile([128, FDIM], F32, name="fidx")
            fidxi = fld.tile([128, FDIM], I16, name="fidxi")
            # wrapped indices: stg[p16, hh*1152 + f*HD + k*16 + j] (f-major)
            # idxw[p, (hh*9 + k)*128 + j*8 + f]
            stg = fld.tile([128, 2 * 8 * HD], I16, name="idxstg")
            idxw = fld.tile([128, 2 * 8 * HD], I16, name="idxw")

            srg = {}

            def pass1(hh):
                for ax in ("y", "x"):
                    s, r, g, i0, cc = (f3(f"s_{ax}"), f3(f"r_{ax}"),
                                       f3(f"g_{ax}"), f3(f"i0_{ax}"),
                                       f3(f"c_{ax}"))
                    base3 = yb3 if ax == "y" else xb3
                    nc.vector.tensor_tensor(
                        f3c(s, hh), offT4[:, 0 if ax == "y" else 1,
                                          hh * CPH:(hh + 1) * CPH],
                        base3[:, hh * CPH:(hh + 1) * CPH], VA.add)
                    nc.vector.tensor_scalar(fsl(r, hh), fsl(s, hh), MAGIC,
                                            -MAGIC, VA.add, VA.add)
                    nc.vector.tensor_tensor(fsl(g, hh), fsl(r, hh), fsl(s, hh),
                                            VA.is_gt)
                    nc.vector.tensor_tensor(fsl(i0, hh), fsl(r, hh), fsl(g, hh),
                                            VA.subtract)
                    nc.vector.tensor_scalar(fsl(cc, hh), fsl(i0, hh), 0.0,
                                            float(H - 2), VA.max, VA.min)
                    srg[ax] = (s, i0, cc)
                # fidx[:, hh*HD + k*16 + j] = cy*W + cx  (h-major, k-blocks)
                out_ap = _ap(fidx, hh * HD,
                             [[FDIM, 128], [CPH, K2], [1, CPH]])
                in_y = _ap(f3("c_y"), hh * HD, [[FDIM, 128], [1, K2], [K2, CPH]])
                in_x = _ap(f3("c_x"), hh * HD, [[FDIM, 128], [1, K2], [K2, CPH]])
                nc.vector.scalar_tensor_tensor(out_ap, in_y, float(W), in_x,
                                               VA.mult, VA.add)
                nc.vector.tensor_copy(fsl(fidxi, hh), fsl(fidx, hh))

            def wrap(hh, engines):
                # collapse 128->16 partitions: 8 small DMAs, split between
                # HWDGE issuers and Pool's software DGE (idle before gathers)
                for f in range(8):
                    eng = engines[f % len(engines)]
                    eng.dma_start(
                        stg[0:16, hh * 8 * HD + f * HD:
                            hh * 8 * HD + (f + 1) * HD],
                        fidxi[16 * f:16 * (f + 1), hh * HD:(hh + 1) * HD])
                # in-partition transpose (f, kj) -> (kj, f), two pipelined
                # pieces so each only waits its half of the collapse DMAs
                for piece in range(2):
                    o_ap = _ap(idxw, hh * 8 * HD + piece * 4,
                               [[2 * 8 * HD, 16], [128, K2], [8, CPH], [1, 4]])
                    i_ap = _ap(stg, hh * 8 * HD + piece * 4 * HD,
                               [[2 * 8 * HD, 16], [CPH, K2], [1, CPH], [HD, 4]])
                    if hh == 0:
                        (nc.scalar.copy if piece == 0
                         else nc.vector.tensor_copy)(o_ap, i_ap)
                    else:
                        nc.vector.tensor_copy(o_ap, i_ap)
                # replicate to all 128 partitions (7 parallel DMAs)
                for f in range(1, 8):
                    eng = engines[f % len(engines)]
                    eng.dma_start(idxw[16 * f:16 * (f + 1),
                                       hh * 8 * HD:(hh + 1) * 8 * HD],
                                  idxw[0:16, hh * 8 * HD:(hh + 1) * 8 * HD])

            # corner-weight pack, dup-paired for DVE 2x broadcast reads:
            # col2 = (k*128 + c*4 + corner)*2 + d, corner order TL,TR,BL,BR
            wpack2 = fld.tile([128, 2 * 4 * FDIM], BF16, name="wpack2")

            def pass2(hh):
                WW = {}
                for ax in ("y", "x"):
                    s, i0, cc = srg[ax]
                    fr, v0, v1, t2 = (f3(f"fr_{ax}"), f3(f"v0_{ax}"),
                                      f3(f"v1_{ax}"), f3(f"t2_{ax}"))
                    nc.vector.tensor_tensor(fsl(fr, hh), fsl(s, hh),
                                            fsl(i0, hh), VA.subtract)
                    nc.vector.tensor_scalar(fsl(v0, hh), fsl(i0, hh), 0.0,
                                            None, VA.is_ge)
                    nc.vector.tensor_scalar(fsl(t2, hh), fsl(i0, hh),
                                            float(H - 1), None, VA.is_le)
                    nc.vector.tensor_tensor(fsl(v0, hh), fsl(v0, hh),
                                            fsl(t2, hh), VA.mult)
                    nc.vector.tensor_scalar(fsl(v1, hh), fsl(i0, hh), -1.0,
                                            None, VA.is_ge)
                    nc.vector.tensor_scalar(fsl(t2, hh), fsl(i0, hh),
                                            float(H - 2), None, VA.is_le)
                    nc.vector.tensor_tensor(fsl(v1, hh), fsl(v1, hh),
                                            fsl(t2, hh), VA.mult)
                    w1v, w0v = f3(f"w1v_{ax}"), f3(f"w0v_{ax}")
                    nc.vector.tensor_tensor(fsl(w1v, hh), fsl(fr, hh),
                                            fsl(v1, hh), VA.mult)
                    nc.vector.tensor_scalar(fsl(w0v, hh), fsl(fr, hh), -1.0,
                                            1.0, VA.mult, VA.add)
                    nc.vector.tensor_tensor(fsl(w0v, hh), fsl(w0v, hh),
                                            fsl(v0, hh), VA.mult)
                    dif = f3(f"dif_{ax}")
                    nc.vector.tensor_tensor(fsl(dif, hh), fsl(cc, hh),
                                            fsl(i0, hh), VA.subtract)
                    eq0, eqP, eqM = (f3(f"eq0_{ax}"), f3(f"eqP_{ax}"),
                                     f3(f"eqM_{ax}"))
                    nc.vector.tensor_scalar(fsl(eq0, hh), fsl(dif, hh), 0.0,
                                            None, VA.is_equal)
                    nc.vector.tensor_scalar(fsl(eqP, hh), fsl(dif, hh), 1.0,
                                            None, VA.is_equal)
                    nc.vector.tensor_scalar(fsl(eqM, hh), fsl(dif, hh), -1.0,
                                            None, VA.is_equal)
                    t1, Wa, Wb = f3(f"t1_{ax}"), f3(f"Wa_{ax}"), f3(f"Wb_{ax}")
                    nc.vector.tensor_tensor(fsl(Wa, hh), fsl(w0v, hh),
                                            fsl(eq0, hh), VA.mult)
                    nc.vector.tensor_tensor(fsl(t1, hh), fsl(w1v, hh),
                                            fsl(eqP, hh), VA.mult)
                    nc.vector.tensor_tensor(fsl(Wa, hh), fsl(Wa, hh),
                                            fsl(t1, hh), VA.add)
                    nc.vector.tensor_tensor(fsl(Wb, hh), fsl(w1v, hh),
                                            fsl(eq0, hh), VA.mult)
                    nc.vector.tensor_tensor(fsl(t1, hh), fsl(w0v, hh),
                                            fsl(eqM, hh), VA.mult)
                    nc.vector.tensor_tensor(fsl(Wb, hh), fsl(Wb, hh),
                                            fsl(t1, hh), VA.add)
                    WW[ax] = (Wa, Wb)
                # products -> wpack2 (k-major quad layout, dup-paired)
                for co, (wy_t, wx_t) in enumerate(
                        ((WW["y"][0], WW["x"][0]), (WW["y"][0], WW["x"][1]),
                         (WW["y"][1], WW["x"][0]), (WW["y"][1], WW["x"][1]))):
                    o_ap = _ap(wpack2, hh * 128 + co * 2,
                               [[8 * FDIM, 128], [256, K2], [8, CPH], [1, 2]])
                    iy = _ap(wy_t, hh * HD,
                             [[FDIM, 128], [1, K2], [K2, CPH], [0, 2]])
                    ix = _ap(wx_t, hh * HD,
                             [[FDIM, 128], [1, K2], [K2, CPH], [0, 2]])
                    nc.vector.tensor_tensor(o_ap, iy, ix, VA.mult)

            def wides_g(h, k, pool, tag="dgw"):
                dgs = []
                for q in range(2):
                    if tag is None:
                        dgw = pool.tile([128, 32 * 128], BF16,
                                        name=f"dl{h}_{k}_{q}")
                    else:
                        dgw = pool.tile([128, 32 * 128], BF16,
                                        name=f"d{h}_{k}_{q}", tag=tag)
                    col0 = k * 128 + (h * CPH + q * 8) * 4
                    in2 = _ap(wpack2, 2 * col0,
                              [[2 * 4 * FDIM, 128], [2, 32], [0, 64], [1, 2]])
                    nc.vector.tensor_tensor(dgw[:, :], idwide[:, :], in2,
                                            VA.mult)
                    dgs.append(dgw)
                return dgs

            # ---- h=0 index path, then the rest staged around it ----
            pass1(0)
            wrap(0, [nc.sync, nc.scalar, nc.gpsimd, nc.scalar])
            pass2(0)
            # prebuild the first slots' diag tiles so the earliest matmuls
            # don't wait behind the h=1 field work on DVE
            prebuilt = {(0, 0): wides_g(0, 0, dgp), (0, 1): wides_g(0, 1, dgp)}
            for cg in range(4, 8):
                offconv(cg)
            pass1(1)
            wrap(1, [nc.sync])
            # more h=0 wides ahead of the (slack-rich) h=1 weight pass; the
            # dgw pool rotation paces these against the consuming matmuls
            for kq in range(2, 6):
                prebuilt[(0, kq)] = wides_g(0, kq, dgp)
            pass2(1)
            for kq in range(6, 9):
                prebuilt[(0, kq)] = wides_g(0, kq, dgp)
            prebuilt[(1, 0)] = wides_g(1, 0, dgp)
            pso_cm.__exit__(None, None, None)

            # ---- gather + diag-matmul accumulate + k-interleaved GEMM ----
            xq0 = xq_in[:, :]
            xq_ap = bass.AP(tensor=xq0.tensor, offset=xq0.offset,
                            ap=[[QE, HW], [1, QE]])
            psp_cm = tc.tile_pool(name="ps", bufs=4, space="PSUM")
            psp = psp_cm.__enter__()
            pso2_cm = tc.tile_pool(name="pso2", bufs=1, space="PSUM")
            pso2 = pso2_cm.__enter__()

            for h in range(NH):
                ot_ps = pso2.tile([128, CPH * 128], F32, name=f"ot{h}", tag="otps")

                def copies(kp, stp):
                    st_sb = stb.tile([128, CPH * 128], BF16,
                                     name=f"st{h}_{kp}", tag="st")
                    for qq in range(4):
                        # k=7's copies run during the tail: use DVE (idle by
                        # then) so they don't serialize with the quarter
                        # copies on ACT
                        if last_h and kp == K2 - 2:
                            nc.vector.tensor_copy(
                                st_sb[:, qq * 512:(qq + 1) * 512], stp[qq][:, :])
                        else:
                            nc.scalar.copy(st_sb[:, qq * 512:(qq + 1) * 512],
                                           stp[qq][:, :])
                    return st_sb

                def gemm(kp, st_sb, jbs=range(4)):
                    # full-PSUM-bank output regions (512 cols fp32): one
                    # open accumulation group per bank across k=0..8
                    for jb in jbs:
                        nc.tensor.matmul(
                            ot_ps[:, jb * 512:(jb + 1) * 512],
                            wmain_sb[:, kp * O:(kp + 1) * O],
                            st_sb[:, jb * 512:(jb + 1) * 512],
                            start=(kp == 0), stop=(kp == K2 - 1))

                def diag_mms(k, gt, dgs, qq, st_ps):
                    for j4 in range(4):
                        j = qq * 4 + j4
                        dg = dgs[j // 8]
                        blk = (j % 8) * 4
                        for co in range(4):
                            nc.tensor.matmul(
                                st_ps[:, j4 * 128:(j4 + 1) * 128],
                                gt[:, j, co * C:(co + 1) * C],
                                dg[:, (blk + co) * 128:(blk + co + 1) * 128],
                                start=(co == 0), stop=(co == 3))

                last_h = (h == NH - 1)
                # pre-build the final slot's diag tiles in dedicated buffers
                # so the tail never waits on DVE or dgw-buffer rotation
                dgs_last = wides_g(h, K2 - 1, cst, tag=None) if last_h else None

                ot_sb = otb.tile([128, CPH * 128], BF16, name=f"ots{h}", tag="ots")
                pend = []   # [(k, st_ps quarters)] awaiting copies
                gpend = []  # [(k, st_sb)] awaiting GEMM
                for k in range(K2):
                    last_slot = last_h and k == K2 - 1
                    base = (h * K2 + k) * 128
                    if not last_slot:
                        gt = gth.tile([128, CPH, QE], BF16,
                                      name=f"g{h}_{k}", tag="gath")
                        # SWDGE ring caps a gather at 1024 descriptors
                        for g2 in range(2):
                            nc.gpsimd.dma_gather(
                                gt[:, g2 * 8:(g2 + 1) * 8, :], xq_ap,
                                idxw[:, base + g2 * 64:base + (g2 + 1) * 64],
                                1024, 1024, QE)
                    else:
                        # last slot: two half tiles so matmuls start on the
                        # first half while the second is still in flight
                        gts = []
                        for g2 in range(2):
                            gth_t = gth.tile([128, 8, QE], BF16,
                                             name=f"g{h}_{k}_{g2}", tag="gath")
                            nc.gpsimd.dma_gather(
                                gth_t[:, :, :], xq_ap,
                                idxw[:, base + g2 * 64:base + (g2 + 1) * 64],
                                1024, 1024, QE)
                            gts.append(gth_t)

                    # drain the pipeline: copies for k-1, GEMM for k-2
                    for kp, stp in pend:
                        gpend.append((kp, copies(kp, stp)))
                    pend = []
                    while len(gpend) > 1:
                        gemm(*gpend.pop(0))

                    if last_slot:
                        dgs = dgs_last
                    elif (h, k) in prebuilt:
                        dgs = prebuilt.pop((h, k))
                    else:
                        dgs = wides_g(h, k, dgp)
                    if not last_slot:
                        st_q = []
                        for qq in range(4):
                            st_ps = psp.tile([128, 4 * 128], F32,
                                             name=f"sp{h}_{k}_{qq}", tag="stps")
                            diag_mms(k, gt, dgs, qq, st_ps)
                            st_q.append(st_ps)
                        pend.append((k, st_q))
                    else:
                        # tail: k-1's GEMM interleaves after the first
                        # quarters so PE never waits on the just-issued
                        # copies. k=8's stop-GEMMs must follow k-1's in PE
                        # program order (PSUM stop = last write per bank).
                        st_sb = stb.tile([128, CPH * 128], BF16,
                                         name=f"st{h}_{k}", tag="st")

                        def act_store(half):
                            sl = slice(half * 1024, (half + 1) * 1024)
                            nc.scalar.activation(
                                ot_sb[:, sl], ot_ps[:, sl],
                                mybir.ActivationFunctionType.Identity,
                                bias=biaso_sb[:, 0:1])
                            nc.sync.dma_start(
                                out_dram[:, h * CPH * 128 + half * 1024:
                                         h * CPH * 128 + (half + 1) * 1024],
                                ot_sb[:, sl])

                        def act_store_q(jb):
                            sl = slice(jb * 512, (jb + 1) * 512)
                            nc.scalar.activation(
                                ot_sb[:, sl], ot_ps[:, sl],
                                mybir.ActivationFunctionType.Identity,
                                bias=biaso_sb[:, 0:1])
                            nc.sync.dma_start(
                                out_dram[:, h * CPH * 128 + jb * 512:
                                         h * CPH * 128 + (jb + 1) * 512],
                                ot_sb[:, sl])

                        for qq in range(4):
                            gtx = gts[qq // 2]
                            st_ps = psp.tile([128, 4 * 128], F32,
                                             name=f"sp{h}_{k}_{qq}", tag="stps")
                            for j4 in range(4):
                                j = qq * 4 + j4
                                dg = dgs[j // 8]
                                blk = (j % 8) * 4
                                for co in range(4):
                                    nc.tensor.matmul(
                                        st_ps[:, j4 * 128:(j4 + 1) * 128],
                                        gtx[:, j % 8, co * C:(co + 1) * C],
                                        dg[:, (blk + co) * 128:
                                           (blk + co + 1) * 128],
                                        start=(co == 0), stop=(co == 3))
                            if qq % 2 == 0:
                                nc.scalar.copy(
                                    st_sb[:, qq * 512:(qq + 1) * 512],
                                    st_ps[:, :])
                            else:
                                nc.vector.tensor_copy(
                                    st_sb[:, qq * 512:(qq + 1) * 512],
                                    st_ps[:, :])
                            if qq == 1:
                                for kp, st_sb_p in gpend:
                                    gemm(kp, st_sb_p)   # k-1, copies now done
                                gpend = []
                                # finish half 0 entirely while the second
                                # half-gather is still in flight
                                gemm(k, st_sb, jbs=[0, 1])
                                act_store(0)
                            elif qq == 3:
                                gemm(k, st_sb, jbs=[2])
                                act_store_q(2)
                        gemm(k, st_sb, jbs=[3])
                        act_store_q(3)

                if not last_h:
                    # tail for h=0: flush k=8 + act/store in halves
                    for kp, stp in pend:
                        gpend.append((kp, copies(kp, stp)))
                    for kp, st_sb_p in gpend:
                        gemm(kp, st_sb_p)
                    pend, gpend = [], []
                    for half in range(2):
                        sl = slice(half * 1024, (half + 1) * 1024)
                        nc.scalar.activation(
                            ot_sb[:, sl], ot_ps[:, sl],
                            mybir.ActivationFunctionType.Identity,
                            bias=biaso_sb[:, 0:1])
                        nc.sync.dma_start(
                            out_dram[:, h * CPH * 128 + half * 1024:
                                     h * CPH * 128 + (half + 1) * 1024],
                            ot_sb[:, sl])
            pso2_cm.__exit__(None, None, None)
            psp_cm.__exit__(None, None, None)

    nc.compile()
    _split_excess_waits(nc)
    return nc


_NC_CACHE = None


def _get_nc():
    global _NC_CACHE
    if _NC_CACHE is None:
        _NC_CACHE = build_nc()
    return _NC_CACHE


def _host_inputs(x, offset_w, offset_b, weight, bias):
    bf = ml_dtypes.bfloat16
    offw = np.ascontiguousarray(
        offset_w.reshape(18, C, K2).transpose(1, 2, 0).reshape(C, K2 * 18)).astype(bf)
    wmain = np.ascontiguousarray(
        weight.reshape(O, C, K2).transpose(1, 2, 0).reshape(C, K2 * O)).astype(bf)
    biaso = bias.reshape(128, 1).astype(np.float32)
    pi = np.arange(128)
    cc = np.arange(NCH)
    kk = np.arange(K2)
    pix = cc[None, :, None] * 128 + pi[:, None, None]          # [128, 32, 1]
    ob = offset_b.reshape(K2, 2).astype(np.float32)
    ybase = (pix // W - 1 + (kk // 3)[None, None, :]
             + ob[None, None, :, 0]).reshape(128, FDIM).astype(np.float32)
    xbase = (pix % W - 1 + (kk % 3)[None, None, :]
             + ob[None, None, :, 1]).reshape(128, FDIM).astype(np.float32)
    idwide = np.ascontiguousarray(
        np.broadcast_to(np.eye(128, dtype=bf)[:, None, :],
                        (128, 32, 128))).reshape(128, 32 * 128)
    shared = dict(offw=offw, wmain=wmain, biaso=biaso,
                  ybase=ybase, xbase=xbase, idwide=idwide)
    maps = []
    for b in range(B):
        m = dict(shared)
        xb = x[b].reshape(C, H, W)
        P = np.zeros((C, H + 2, GP), np.float32)
        P[:, 1:H + 1, 1:W + 1] = xb
        m["xg"] = P.reshape(C, XG).astype(bf)
        # quad tensor: Q[y*W+x] = [x(y,x,:), x(y,x+1,:), x(y+1,x,:), x(y+1,x+1,:)]
        xp = np.zeros((C, H + 1, W + 1), np.float32)
        xp[:, :H, :W] = xb
        q = np.stack([xp[:, :H, :W], xp[:, :H, 1:W + 1],
                      xp[:, 1:H + 1, :W], xp[:, 1:H + 1, 1:W + 1]], axis=0)
        m["xq"] = np.ascontiguousarray(
            q.transpose(2, 3, 0, 1).reshape(HW, QE)).astype(bf)
        maps.append(m)
    return maps


def kernel(x, offset_w, offset_b, weight, bias):
    from concourse.bass_utils import run_bass_kernel_spmd
    nc = _get_nc()
    in_maps = _host_inputs(np.asarray(x, np.float32), np.asarray(offset_w, np.float32),
                           np.asarray(offset_b, np.float32),
                           np.asarray(weight, np.float32), np.asarray(bias, np.float32))
    res = run_bass_kernel_spmd(nc, in_maps, core_ids=list(range(B)))
    out = np.stack([np.asarray(res.results[b]["out"], np.float32).reshape(O, H, W)
                    for b in range(B)])
    return out
